# revision 1
# baseline (speedup 1.0000x reference)
"""2-layer GCN (GCNConv x2) on 8 Trainium2 NeuronCores via Bass.

Strategy (dst-sharded):
- Nodes sharded into 8 contiguous slices of 31250 (padded to 31360 = 245*128).
- Within each core, nodes are sorted by in-degree and packed into 245 blocks
  of 128 (block b, partition p). Per-block ELL: K_b gather rounds, each an
  indirect DMA pulling 128 rows of 16 floats from a DRAM feature table into
  block tile columns -- landing position encodes the destination, so no
  scatter is needed. A strided DVE reduce sums the K_b slots per node.
- Feature tables (dis * h per node) are exchanged with AllGather (2MB/rank).
- Self-loops are the locally available own-slice rows (added once, scaled).
- A_hat = D^-1/2 (A+I) D^-1/2 is factored as pre-scale (table rows carry
  dis*h) and post-scale (dis * aggregate), so no per-edge norm is needed.
"""
import os
import sys
import types

sys.path.insert(0, "/opt/trn_rl_repo")

import numpy as np

N = 250000
E = 4000000
IN_F, HID, OUT = 18, 16, 1
NCORES = 8
NSLICE = N // NCORES            # 31250
BLKS = (NSLICE + 127) // 128    # 245
NPAD = BLKS * 128               # 31360
P = 128

LAST_RESULTS = None             # test.py reads exec_time_ns from here


def _install_shims():
    """Make run_bass_kernel_spmd(trace=True) work in this container."""
    try:
        import antenv.axon_hooks  # noqa: F401
    except ImportError:
        import antenv
        mod = types.ModuleType("antenv.axon_hooks")
        _hook = [None]
        mod.set_axon_ntff_profile_hook = lambda h: _hook.__setitem__(0, h)
        mod.get_axon_ntff_profile_hook = lambda: _hook[0]
        sys.modules["antenv.axon_hooks"] = mod
        antenv.axon_hooks = mod
        try:
            from trn_agent_boot import trn_boot
            mod.set_axon_ntff_profile_hook(
                trn_boot._ntff_profile_via_ctypes("/opt/axon/libaxon_pjrt.so")
            )
        except Exception:
            pass
    from concourse import bass_utils
    bass_utils.upload_artifacts = lambda tmpdir: tmpdir


def _host_prep(x, edge_index, W1, b1, W2, b2):
    src = np.asarray(edge_index[0], dtype=np.int64).astype(np.int32)
    dst = np.asarray(edge_index[1], dtype=np.int64).astype(np.int32)
    x = np.asarray(x, dtype=np.float32)

    deg_in = np.bincount(dst, minlength=N).astype(np.int64)   # without self loop

    # per-core degree-ascending rank of each node
    rank = np.empty(N, dtype=np.int64)
    orders = []
    for c in range(NCORES):
        lo, hi = c * NSLICE, (c + 1) * NSLICE
        order = np.argsort(deg_in[lo:hi], kind="stable")      # ascending
        orders.append(order)
        rank[lo + order] = np.arange(NSLICE)
    owner = np.arange(N) // NSLICE
    table_row = owner * NPAD + rank                           # global table row

    # common per-block slot counts K_b (max over cores, +block padding)
    K = np.zeros(BLKS, dtype=np.int64)
    for c in range(NCORES):
        lo = c * NSLICE
        ds = deg_in[lo + orders[c]]                           # ascending
        ds_pad = np.concatenate([ds, np.zeros(NPAD - NSLICE, np.int64)])
        K = np.maximum(K, ds_pad.reshape(BLKS, P).max(axis=1))
    K = np.maximum(K, 1)
    off = np.concatenate([[0], np.cumsum(K)]).astype(np.int64)
    C_idx = int(off[-1])

    # place each edge: sorted by dst, k-th in-edge of node d goes to
    # column off[b]+k on partition p, where rank[d] = b*128+p
    es = np.argsort(dst, kind="stable")
    dsts = dst[es]
    srcs = src[es]
    run_first = np.searchsorted(dsts, np.arange(N))           # first pos per node
    k_arr = np.arange(E, dtype=np.int64) - run_first[dsts]
    c_arr = dsts // NSLICE
    r_arr = rank[dsts]
    b_arr = r_arr // P
    p_arr = r_arr % P
    col_arr = off[b_arr] + k_arr
    trow_arr = table_row[srcs]

    dead_row = (np.arange(NCORES) * NPAD + NPAD - 1).astype(np.int32)
    idx_all = np.broadcast_to(
        dead_row[:, None, None], (NCORES, P, C_idx)
    ).astype(np.int32).copy()
    idx_all[c_arr, p_arr, col_arr] = trow_arr.astype(np.int32)

    # per-core tensors
    in_maps = []
    for c in range(NCORES):
        lo = c * NSLICE
        order = orders[c]
        xT = np.zeros((IN_F, NPAD), dtype=np.float32)
        xT[:, :NSLICE] = x[lo + order].T
        deg_t = np.ones(NPAD, dtype=np.float32)
        deg_t[:NSLICE] = deg_in[lo + order].astype(np.float32) + 1.0
        deg_t = deg_t.reshape(BLKS, P).T.copy()               # [128, 245]
        in_maps.append({
            "xT": xT,
            "degt": deg_t,
            "idx": idx_all[c],
            "W1": np.asarray(W1, dtype=np.float32),
            "W2r": np.tile(np.asarray(W2, np.float32).reshape(1, HID), (P, 1)),
            "b1r": np.tile(np.asarray(b1, np.float32).reshape(1, HID), (P, 1)),
            "b2r": np.full((P, 1), np.float32(np.asarray(b2).reshape(-1)[0])),
            "lmask": np.where(np.arange(P) < NSLICE - (BLKS - 1) * P, 1.0, 0.0)
                       .astype(np.float32).reshape(P, 1),
        })
    meta = {"K": K.tolist(), "off": off.tolist(), "C_idx": C_idx,
            "orders": orders}
    return in_maps, meta


def _build_nc(K, C_idx):
    from concourse import bass, bacc, mybir
    import concourse.tile as tile

    nc = bacc.Bacc("TRN2", target_bir_lowering=False, debug=False,
                   num_devices=NCORES)
    f32 = mybir.dt.float32
    xT_d = nc.dram_tensor("xT", [IN_F, NPAD], f32, kind="ExternalInput")
    degt_d = nc.dram_tensor("degt", [P, BLKS], f32, kind="ExternalInput")
    idx_d = nc.dram_tensor("idx", [P, C_idx], mybir.dt.int32, kind="ExternalInput")
    W1_d = nc.dram_tensor("W1", [IN_F, HID], f32, kind="ExternalInput")
    W2r_d = nc.dram_tensor("W2r", [P, HID], f32, kind="ExternalInput")
    b1r_d = nc.dram_tensor("b1r", [P, HID], f32, kind="ExternalInput")
    b2r_d = nc.dram_tensor("b2r", [P, 1], f32, kind="ExternalInput")
    lmask_d = nc.dram_tensor("lmask", [P, 1], f32, kind="ExternalInput")
    out_d = nc.dram_tensor("o", [NPAD], f32, kind="ExternalOutput")

    FB = BLKS * HID  # 3920 free cols for [p, (b f)] layouts
    NDEAD = NPAD - NSLICE          # 110 dead slots (block 244, p >= 18)
    DEADP = NSLICE - (BLKS - 1) * P  # first dead partition in last block (18)

    with tile.TileContext(nc) as tc:
        with (
            tc.tile_pool(name="const", bufs=1) as cp,
            tc.tile_pool(name="xp", bufs=2) as xp,
            tc.tile_pool(name="ps", bufs=2, space="PSUM") as psp,
            tc.tile_pool(name="wk", bufs=1) as wk,
            tc.tile_pool(name="gth", bufs=4) as gth,
            tc.tile_pool(name="dram", bufs=1, space="DRAM") as dr,
        ):
            W1s = cp.tile([IN_F, HID], f32)
            nc.sync.dma_start(out=W1s[:], in_=W1_d[:])
            W2s = cp.tile([P, HID], f32)
            nc.sync.dma_start(out=W2s[:], in_=W2r_d[:])
            b1s = cp.tile([P, HID], f32)
            nc.sync.dma_start(out=b1s[:], in_=b1r_d[:])
            b2s = cp.tile([P, 1], f32)
            nc.sync.dma_start(out=b2s[:], in_=b2r_d[:])
            lmask = cp.tile([P, 1], f32)
            nc.sync.dma_start(out=lmask[:], in_=lmask_d[:])
            degs = cp.tile([P, BLKS], f32)
            nc.sync.dma_start(out=degs[:], in_=degt_d[:])
            idxs = cp.tile([P, C_idx], mybir.dt.int32)
            nc.sync.dma_start(out=idxs[:], in_=idx_d[:])

            dis = cp.tile([P, BLKS], f32)
            nc.vector.reciprocal(out=dis[:], in_=degs[:])
            nc.scalar.activation(out=dis[:], in_=dis[:],
                                 func=mybir.ActivationFunctionType.Sqrt)

            ag1in = dr.tile([NPAD, HID], f32)
            table1 = dr.tile([NCORES * NPAD, HID], f32, addr_space="Shared")
            ag2in = dr.tile([NPAD, HID], f32)
            table2 = dr.tile([NCORES * NPAD, HID], f32, addr_space="Shared")

            # ---- phase A: htab = dis * (x @ W1), rank-ordered [p, (b f)] ----
            htab = wk.tile([P, FB], f32)
            CHUNK = 32
            for piece in range((BLKS + CHUNK - 1) // CHUNK):
                b0 = piece * CHUNK
                nb = min(CHUNK, BLKS - b0)
                xpc = xp.tile([IN_F, CHUNK * P], f32, tag="xpc")
                nc.sync.dma_start(out=xpc[:, : nb * P],
                                  in_=xT_d[:, b0 * P : (b0 + nb) * P])
                pst = psp.tile([P, CHUNK * HID], f32, tag="pst")
                for j in range(nb):
                    nc.tensor.matmul(
                        out=pst[:, j * HID : (j + 1) * HID],
                        lhsT=xpc[:, j * P : (j + 1) * P],
                        rhs=W1s[:],
                        start=True, stop=True,
                    )
                # scale by dis while copying PSUM -> SBUF
                dis_b = dis[:, b0 : b0 + nb].rearrange("p (b one) -> p b one", one=1)
                nc.vector.tensor_tensor(
                    out=htab[:, b0 * HID : (b0 + nb) * HID],
                    in0=pst[:, : nb * HID].rearrange("p (b f) -> p b f", f=HID),
                    in1=dis_b.to_broadcast([P, nb, HID]),
                    op=mybir.AluOpType.mult,
                )
            nc.sync.dma_start(
                out=ag1in[:].rearrange("(b p) f -> p b f", p=P),
                in_=htab[:].rearrange("p (b f) -> p b f", f=HID),
            )
            nc.gpsimd.collective_compute(
                "AllGather", mybir.AluOpType.bypass,
                replica_groups=[list(range(NCORES))],
                ins=[ag1in.opt()], outs=[table1.opt()],
            )

            def aggregate(table, accname):
                acc = wk.tile([P, FB], f32, name=accname)
                for b in range(BLKS):
                    kb = K[b]
                    bt = gth.tile([P, int(max(K)) * HID], f32, tag="bt")
                    for k in range(kb):
                        col = OFF[b] + k
                        nc.gpsimd.indirect_dma_start(
                            out=bt[:, k * HID : (k + 1) * HID],
                            out_offset=None,
                            in_=table[:, :],
                            in_offset=bass.IndirectOffsetOnAxis(
                                ap=idxs[:, col : col + 1], axis=0
                            ),
                        )
                    src = bt[:, : kb * HID].rearrange(
                        "p (k f) -> p f k", f=HID
                    )
                    nc.vector.reduce_sum(
                        out=acc[:, b * HID : (b + 1) * HID],
                        in_=src, axis=mybir.AxisListType.X,
                    )
                return acc

            OFF = [0]
            for kb in K:
                OFF.append(OFF[-1] + kb)

            disB = dis[:].rearrange("p (b one) -> p b one", one=1)

            # ---- layer 1 ----
            acc1 = aggregate(table1, "acc1")
            nc.vector.tensor_add(out=acc1[:], in0=acc1[:], in1=htab[:])
            # y1 = acc1 * dis ; h1 = relu(y1 + b1) ; htab2 = dis * h1
            nc.vector.tensor_tensor(
                out=acc1[:],
                in0=acc1[:].rearrange("p (b f) -> p b f", f=HID),
                in1=disB.to_broadcast([P, BLKS, HID]),
                op=mybir.AluOpType.mult,
            )
            nc.vector.tensor_tensor(
                out=acc1[:],
                in0=acc1[:].rearrange("p (b f) -> p b f", f=HID),
                in1=b1s[:].rearrange("p (one f) -> p one f", one=1).to_broadcast([P, BLKS, HID]),
                op=mybir.AluOpType.add,
            )
            nc.scalar.activation(out=acc1[:], in_=acc1[:],
                                 func=mybir.ActivationFunctionType.Relu)
            htab2 = wk.tile([P, FB], f32)
            nc.vector.tensor_tensor(
                out=htab2[:],
                in0=acc1[:].rearrange("p (b f) -> p b f", f=HID),
                in1=disB.to_broadcast([P, BLKS, HID]),
                op=mybir.AluOpType.mult,
            )
            # zero the dead slots (last block, partitions >= DEADP)
            nc.vector.tensor_tensor(
                out=htab2[:, (BLKS - 1) * HID :],
                in0=htab2[:, (BLKS - 1) * HID :],
                in1=lmask[:].to_broadcast([P, HID]),
                op=mybir.AluOpType.mult,
            )

            nc.sync.dma_start(
                out=ag2in[:].rearrange("(b p) f -> p b f", p=P),
                in_=htab2[:].rearrange("p (b f) -> p b f", f=HID),
            )
            nc.gpsimd.collective_compute(
                "AllGather", mybir.AluOpType.bypass,
                replica_groups=[list(range(NCORES))],
                ins=[ag2in.opt()], outs=[table2.opt()],
            )

            # ---- layer 2 ----
            acc2 = aggregate(table2, "acc2")
            nc.vector.tensor_add(out=acc2[:], in0=acc2[:], in1=htab2[:])
            nc.vector.tensor_tensor(
                out=acc2[:],
                in0=acc2[:].rearrange("p (b f) -> p b f", f=HID),
                in1=disB.to_broadcast([P, BLKS, HID]),
                op=mybir.AluOpType.mult,
            )
            nc.vector.tensor_tensor(
                out=acc2[:],
                in0=acc2[:].rearrange("p (b f) -> p b f", f=HID),
                in1=W2s[:].rearrange("p (one f) -> p one f", one=1).to_broadcast([P, BLKS, HID]),
                op=mybir.AluOpType.mult,
            )
            y2 = wk.tile([P, BLKS], f32)
            nc.vector.reduce_sum(
                out=y2[:],
                in_=acc2[:].rearrange("p (b f) -> p b f", f=HID),
                axis=mybir.AxisListType.X,
            )
            nc.vector.tensor_tensor(
                out=y2[:],
                in0=y2[:],
                in1=b2s[:].to_broadcast([P, BLKS]),
                op=mybir.AluOpType.add,
            )
            nc.sync.dma_start(
                out=out_d[:].rearrange("(b p) -> p b", p=P),
                in_=y2[:],
            )
    nc.compile()
    return nc


def kernel(x, edge_index, W1, b1, W2, b2):
    global LAST_RESULTS
    _install_shims()
    from concourse.bass_utils import run_bass_kernel_spmd

    in_maps, meta = _host_prep(x, edge_index, W1, b1, W2, b2)
    nc = _build_nc(meta["K"], meta["C_idx"])
    res = run_bass_kernel_spmd(
        nc, in_maps, core_ids=list(range(NCORES)),
        trace=bool(os.environ.get("BASS_TRACE")),
    )
    LAST_RESULTS = res
    out = np.empty((N, 1), dtype=np.float32)
    for c in range(NCORES):
        yc = res.results[c]["o"]            # [NPAD], rank-ordered
        lo = c * NSLICE
        out[lo + meta["orders"][c], 0] = yc[:NSLICE]
    return out



# revision 12
# speedup vs baseline: 1.1755x; 1.1755x over previous
"""2-layer GCN (GCNConv x2) on 8 Trainium2 NeuronCores via Bass.

Strategy (dst-sharded, dma_gather ELL):
- Nodes sharded into 8 contiguous slices of 31250 (padded to 31360 = 245*128).
- Within each core, nodes are sorted by in-degree and packed into 245 blocks
  of 128 (block b, partition p). Per-block ELL: K_b slot columns per block
  (max in-degree + 1 self slot, maxed across cores).
- Feature table: dis*h per node, bf16, packed 8 nodes per 256B "group" row
  ([31360 groups, 128 bf16] globally). AllGathered per layer (1MB/rank).
- Gather: InstDMAGatherAnt (gpsimd.dma_gather) with int16 group indices,
  8 ELL columns (1024 idxs / 1024 descriptors) per instruction -- descriptor
  ring limit. Index j lands at [p=j%128, col=j//128], so landing position
  encodes the destination slot; no scatter needed.
- Each gathered 256B group holds 8 candidate nodes; a per-slot one-hot bf16
  mask (8 lanes) selects the right node (and zeroes padding slots). One
  strided DVE reduce per block sums over (slots x 8 lanes).
- Self-loops are ELL slots pointing at the node's own group/residue: with
  A_hat = D^-1/2 (A+I) D^-1/2 factored as pre-scale (table rows carry dis*h)
  and post-scale (dis * aggregate), the self term dis^2*h is exact.
"""
import os
import sys
import types

sys.path.insert(0, "/opt/trn_rl_repo")

import numpy as np
import ml_dtypes

N = 250000
E = 4000000
IN_F, HID, OUT = 18, 16, 1
NCORES = 8
NSLICE = N // NCORES            # 31250
BLKS = (NSLICE + 127) // 128    # 245
NPAD = BLKS * 128               # 31360
P = 128
GSEG = NPAD // 8                # 3920 groups per core segment
G_ALL = NCORES * GSEG           # 31360 groups globally (< int16 max)

SCMAX = 64                      # ELL columns per super-chunk (gather tile)
ICOLS = 8                       # ELL columns per dma_gather instruction
NQ = 1                          # SWDGE queues

LAST_RESULTS = None             # test.py reads exec_time_ns from here


def _install_shims():
    """Make run_bass_kernel_spmd(trace=True) work in this container."""
    try:
        import antenv.axon_hooks  # noqa: F401
    except ImportError:
        import antenv
        mod = types.ModuleType("antenv.axon_hooks")
        _hook = [None]
        mod.set_axon_ntff_profile_hook = lambda h: _hook.__setitem__(0, h)
        mod.get_axon_ntff_profile_hook = lambda: _hook[0]
        sys.modules["antenv.axon_hooks"] = mod
        antenv.axon_hooks = mod
        try:
            from trn_agent_boot import trn_boot
            mod.set_axon_ntff_profile_hook(
                trn_boot._ntff_profile_via_ctypes("/opt/axon/libaxon_pjrt.so")
            )
        except Exception:
            pass
    from concourse import bass_utils
    bass_utils.upload_artifacts = lambda tmpdir: tmpdir


def _host_prep(x, edge_index, W1, b1, W2, b2):
    src = np.asarray(edge_index[0], dtype=np.int64).astype(np.int32)
    dst = np.asarray(edge_index[1], dtype=np.int64).astype(np.int32)
    x = np.asarray(x, dtype=np.float32)

    deg_in = np.bincount(dst, minlength=N).astype(np.int64)   # without self loop

    # per-core degree-ascending rank of each node
    rank = np.empty(N, dtype=np.int64)
    orders = []
    for c in range(NCORES):
        lo, hi = c * NSLICE, (c + 1) * NSLICE
        order = np.argsort(deg_in[lo:hi], kind="stable")      # ascending
        orders.append(order)
        rank[lo + order] = np.arange(NSLICE)
    owner = np.arange(N) // NSLICE
    table_row = owner * NPAD + rank                           # global table row

    # common per-block slot counts K_b (max over cores) + 1 self slot
    K = np.zeros(BLKS, dtype=np.int64)
    for c in range(NCORES):
        lo = c * NSLICE
        ds = deg_in[lo + orders[c]]                           # ascending
        ds_pad = np.concatenate([ds, np.zeros(NPAD - NSLICE, np.int64)])
        K = np.maximum(K, ds_pad.reshape(BLKS, P).max(axis=1))
    K = K + 1                                                 # self slot
    off = np.concatenate([[0], np.cumsum(K)]).astype(np.int64)
    C_idx = int(off[-1])

    # place each edge: sorted by dst, k-th in-edge of node d goes to
    # column off[b]+k on partition p, where rank[d] = b*128+p
    es = np.argsort(dst, kind="stable")
    dsts = dst[es]
    srcs = src[es]
    run_first = np.searchsorted(dsts, np.arange(N))           # first pos per node
    k_arr = np.arange(E, dtype=np.int64) - run_first[dsts]
    c_arr = dsts // NSLICE
    r_arr = rank[dsts]
    b_arr = r_arr // P
    p_arr = r_arr % P
    col_arr = off[b_arr] + k_arr
    trow_arr = table_row[srcs]

    # per-slot group index + residue + validity, [NCORES, P, C_idx]
    grp_all = np.zeros((NCORES, P, C_idx), dtype=np.int16)
    res_all = np.zeros((NCORES, P, C_idx), dtype=np.int8)
    val_all = np.zeros((NCORES, P, C_idx), dtype=bool)
    grp_all[c_arr, p_arr, col_arr] = (trow_arr // 8).astype(np.int16)
    res_all[c_arr, p_arr, col_arr] = (trow_arr % 8).astype(np.int8)
    val_all[c_arr, p_arr, col_arr] = True

    # self slots: node (c, b, p) real iff rank < NSLICE; its table row is
    # c*NPAD + b*128 + p; self slot at column off[b] + deg
    for c in range(NCORES):
        lo = c * NSLICE
        deg_ord = np.concatenate(
            [deg_in[lo + orders[c]], np.zeros(NPAD - NSLICE, np.int64)]
        )
        ranks = np.arange(NPAD)
        bs, ps = ranks // P, ranks % P
        cols = off[bs] + deg_ord
        rows = c * NPAD + ranks
        real = ranks < NSLICE
        grp_all[c, ps[real], cols[real]] = (rows[real] // 8).astype(np.int16)
        res_all[c, ps[real], cols[real]] = (rows[real] % 8).astype(np.int8)
        val_all[c, ps[real], cols[real]] = True

    # block-aligned super-chunks of <= SCMAX columns
    chunks = []
    cb0 = 0
    while cb0 < BLKS:
        cb1 = cb0
        while cb1 < BLKS and (off[cb1 + 1] - off[cb0]) <= SCMAX:
            cb1 += 1
        assert cb1 > cb0, f"block {cb0} has K={K[cb0]} > SCMAX={SCMAX}"
        chunks.append((cb0, cb1, int(off[cb0]), int(off[cb1])))
        cb0 = cb1

    # per-core tensors
    in_maps = []
    for c in range(NCORES):
        lo = c * NSLICE
        order = orders[c]
        xT = np.zeros((IN_F, NPAD), dtype=np.float32)
        xT[:, :NSLICE] = x[lo + order].T
        deg_t = np.ones(NPAD, dtype=np.float32)
        deg_t[:NSLICE] = deg_in[lo + order].astype(np.float32) + 1.0
        deg_t = deg_t.reshape(BLKS, P).T.copy()               # [128, 245]

        # idx wrap: element j = s*128+p of any column-aligned instruction is
        # at [j%16, s*8 + p//16]; replicated across the 8 16-partition groups
        idxg = (
            grp_all[c].reshape(8, 16, C_idx).transpose(1, 2, 0).reshape(16, C_idx * 8)
        )
        idxg = np.tile(idxg, (8, 1)).astype(np.int16)

        # mask: [p, s*8 + r] one-hot bf16 (zero for padding slots)
        onehot = (
            np.arange(8, dtype=np.int8)[None, None, :] == res_all[c][:, :, None]
        ) & val_all[c][:, :, None]
        maskt = onehot.reshape(P, C_idx * 8).astype(ml_dtypes.bfloat16)

        in_maps.append({
            "xT": xT,
            "degt": deg_t,
            "idxg": idxg,
            "maskt": maskt,
            "W1": np.asarray(W1, dtype=np.float32),
            "W2r": np.tile(np.asarray(W2, np.float32).reshape(1, HID), (P, 1)),
            "b1r": np.tile(np.asarray(b1, np.float32).reshape(1, HID), (P, 1)),
            "b2r": np.full((P, 1), np.float32(np.asarray(b2).reshape(-1)[0])),
        })
    meta = {"K": K.tolist(), "off": off.tolist(), "C_idx": C_idx,
            "orders": orders, "chunks": chunks}
    return in_maps, meta


def _build_nc(K, C_idx, CHUNKS):
    from concourse import bass, bacc, mybir
    import concourse.tile as tile

    nc = bacc.Bacc("TRN2", target_bir_lowering=False, debug=False,
                   num_devices=NCORES, num_swdge_queues=NQ)
    f32 = mybir.dt.float32
    bf16 = mybir.dt.bfloat16
    i16 = mybir.dt.int16
    xT_d = nc.dram_tensor("xT", [IN_F, NPAD], f32, kind="ExternalInput")
    degt_d = nc.dram_tensor("degt", [P, BLKS], f32, kind="ExternalInput")
    idxg_d = nc.dram_tensor("idxg", [P, C_idx * 8], i16, kind="ExternalInput")
    maskt_d = nc.dram_tensor("maskt", [P, C_idx * 8], bf16, kind="ExternalInput")
    W1_d = nc.dram_tensor("W1", [IN_F, HID], f32, kind="ExternalInput")
    W2r_d = nc.dram_tensor("W2r", [P, HID], f32, kind="ExternalInput")
    b1r_d = nc.dram_tensor("b1r", [P, HID], f32, kind="ExternalInput")
    b2r_d = nc.dram_tensor("b2r", [P, 1], f32, kind="ExternalInput")
    out_d = nc.dram_tensor("o", [NPAD], f32, kind="ExternalOutput")

    FB = BLKS * HID  # 3920 free cols for [p, (b f)] layouts

    OFF = [0]
    for kb in K:
        OFF.append(OFF[-1] + kb)

    with tile.TileContext(nc) as tc:
        with (
            tc.tile_pool(name="const", bufs=1) as cp,
            tc.tile_pool(name="xp", bufs=2) as xp,
            tc.tile_pool(name="ps", bufs=2, space="PSUM") as psp,
            tc.tile_pool(name="wk", bufs=1) as wk,
            tc.tile_pool(name="gth", bufs=3) as gth,
            tc.tile_pool(name="ixp", bufs=3) as ixp,
            tc.tile_pool(name="dram", bufs=1, space="DRAM") as dr,
        ):
            W1s = cp.tile([IN_F, HID], f32)
            nc.sync.dma_start(out=W1s[:], in_=W1_d[:])
            W2s = cp.tile([P, HID], f32)
            nc.sync.dma_start(out=W2s[:], in_=W2r_d[:])
            b1s = cp.tile([P, HID], f32)
            nc.sync.dma_start(out=b1s[:], in_=b1r_d[:])
            b2s = cp.tile([P, 1], f32)
            nc.sync.dma_start(out=b2s[:], in_=b2r_d[:])
            degs = cp.tile([P, BLKS], f32)
            nc.sync.dma_start(out=degs[:], in_=degt_d[:])

            dis = cp.tile([P, BLKS], f32)
            nc.vector.reciprocal(out=dis[:], in_=degs[:])
            nc.scalar.activation(out=dis[:], in_=dis[:],
                                 func=mybir.ActivationFunctionType.Sqrt)

            ag1in = dr.tile([GSEG, 128], bf16)
            table1 = dr.tile([G_ALL, 128], bf16, addr_space="Shared")
            ag2in = dr.tile([GSEG, 128], bf16)
            table2 = dr.tile([G_ALL, 128], bf16, addr_space="Shared")

            disB = dis[:].rearrange("p (b one) -> p b one", one=1)

            # ---- phase A: htab = dis * (x @ W1) in bf16, rank order ----
            htab = wk.tile([P, FB], bf16)
            CHUNK = 32
            for piece in range((BLKS + CHUNK - 1) // CHUNK):
                b0 = piece * CHUNK
                nb = min(CHUNK, BLKS - b0)
                xpc = xp.tile([IN_F, CHUNK * P], f32, tag="xpc")
                nc.sync.dma_start(out=xpc[:, : nb * P],
                                  in_=xT_d[:, b0 * P : (b0 + nb) * P])
                pst = psp.tile([P, CHUNK * HID], f32, tag="pst")
                for j in range(nb):
                    nc.tensor.matmul(
                        out=pst[:, j * HID : (j + 1) * HID],
                        lhsT=xpc[:, j * P : (j + 1) * P],
                        rhs=W1s[:],
                        start=True, stop=True,
                    )
                dis_b = dis[:, b0 : b0 + nb].rearrange("p (b one) -> p b one", one=1)
                nc.vector.tensor_tensor(
                    out=htab[:, b0 * HID : (b0 + nb) * HID],
                    in0=pst[:, : nb * HID].rearrange("p (b f) -> p b f", f=HID),
                    in1=dis_b.to_broadcast([P, nb, HID]),
                    op=mybir.AluOpType.mult,
                )
            nc.sync.dma_start(
                out=ag1in[:].rearrange("(b ph) (pl f) -> (ph pl) b f",
                                       ph=16, pl=8, f=HID),
                in_=htab[:].rearrange("p (b f) -> p b f", f=HID),
            )
            nc.gpsimd.collective_compute(
                "AllGather", mybir.AluOpType.bypass,
                replica_groups=[list(range(NCORES))],
                ins=[ag1in.opt()], outs=[table1.opt()],
            )

            qn = [0]

            def aggregate(table, accname):
                acc = wk.tile([P, FB], f32, name=accname)
                for (b0, b1, s0, s1) in CHUNKS:
                    SC = s1 - s0
                    bt = gth.tile([P, SCMAX * 128], bf16, tag="bt")
                    ixt = ixp.tile([P, SCMAX * 8], i16, tag="ixt")
                    nc.sync.dma_start(out=ixt[:, : SC * 8],
                                      in_=idxg_d[:, s0 * 8 : s1 * 8])
                    mkt = ixp.tile([P, SCMAX * 8], bf16, tag="mkt")
                    nc.sync.dma_start(out=mkt[:, : SC * 8],
                                      in_=maskt_d[:, s0 * 8 : s1 * 8])
                    # gather in <= ICOLS column pieces (descriptor ring limit)
                    for c0 in range(0, SC, ICOLS):
                        c1 = min(c0 + ICOLS, SC)
                        nidx = (c1 - c0) * P
                        nc.gpsimd.dma_gather(
                            out_ap=bt[:, c0 * 128 : c1 * 128].rearrange(
                                "p (s e) -> p s e", e=128
                            ),
                            in_ap=table[:, :],
                            idxs_ap=ixt[:, c0 * 8 : c1 * 8],
                            num_idxs=nidx,
                            num_idxs_reg=nidx,
                            elem_size=128,
                            queue_num=qn[0] % NQ,
                        )
                        qn[0] += 1
                    # select the target node in each gathered 8-group
                    nc.vector.tensor_tensor(
                        out=bt[:, : SC * 128].rearrange(
                            "p (s r f) -> p s r f", r=8, f=HID
                        ),
                        in0=bt[:, : SC * 128].rearrange(
                            "p (s r f) -> p s r f", r=8, f=HID
                        ),
                        in1=mkt[:, : SC * 8].rearrange(
                            "p (s r one) -> p s r one", r=8, one=1
                        ).to_broadcast([P, SC, 8, HID]),
                        op=mybir.AluOpType.mult,
                    )
                    for b in range(b0, b1):
                        kb = K[b]
                        o = OFF[b] - s0
                        nc.vector.reduce_sum(
                            out=acc[:, b * HID : (b + 1) * HID],
                            in_=bt[:, o * 128 : (o + kb) * 128].rearrange(
                                "p (kr f) -> p f kr", f=HID
                            ),
                            axis=mybir.AxisListType.X,
                        )
                return acc

            # ---- layer 1 (self term included via ELL self slots) ----
            acc1 = aggregate(table1, "acc1")
            # y1 = acc1 * dis ; h1 = relu(y1 + b1) ; htab2 = dis * h1
            nc.vector.tensor_tensor(
                out=acc1[:],
                in0=acc1[:].rearrange("p (b f) -> p b f", f=HID),
                in1=disB.to_broadcast([P, BLKS, HID]),
                op=mybir.AluOpType.mult,
            )
            nc.vector.tensor_tensor(
                out=acc1[:],
                in0=acc1[:].rearrange("p (b f) -> p b f", f=HID),
                in1=b1s[:].rearrange("p (one f) -> p one f", one=1).to_broadcast([P, BLKS, HID]),
                op=mybir.AluOpType.add,
            )
            nc.scalar.activation(out=acc1[:], in_=acc1[:],
                                 func=mybir.ActivationFunctionType.Relu)
            htab2 = wk.tile([P, FB], bf16)
            nc.vector.tensor_tensor(
                out=htab2[:],
                in0=acc1[:].rearrange("p (b f) -> p b f", f=HID),
                in1=disB.to_broadcast([P, BLKS, HID]),
                op=mybir.AluOpType.mult,
            )

            nc.sync.dma_start(
                out=ag2in[:].rearrange("(b ph) (pl f) -> (ph pl) b f",
                                       ph=16, pl=8, f=HID),
                in_=htab2[:].rearrange("p (b f) -> p b f", f=HID),
            )
            nc.gpsimd.collective_compute(
                "AllGather", mybir.AluOpType.bypass,
                replica_groups=[list(range(NCORES))],
                ins=[ag2in.opt()], outs=[table2.opt()],
            )

            # ---- layer 2 ----
            acc2 = aggregate(table2, "acc2")
            nc.vector.tensor_tensor(
                out=acc2[:],
                in0=acc2[:].rearrange("p (b f) -> p b f", f=HID),
                in1=disB.to_broadcast([P, BLKS, HID]),
                op=mybir.AluOpType.mult,
            )
            nc.vector.tensor_tensor(
                out=acc2[:],
                in0=acc2[:].rearrange("p (b f) -> p b f", f=HID),
                in1=W2s[:].rearrange("p (one f) -> p one f", one=1).to_broadcast([P, BLKS, HID]),
                op=mybir.AluOpType.mult,
            )
            y2 = wk.tile([P, BLKS], f32)
            nc.vector.reduce_sum(
                out=y2[:],
                in_=acc2[:].rearrange("p (b f) -> p b f", f=HID),
                axis=mybir.AxisListType.X,
            )
            nc.vector.tensor_tensor(
                out=y2[:],
                in0=y2[:],
                in1=b2s[:].to_broadcast([P, BLKS]),
                op=mybir.AluOpType.add,
            )
            nc.sync.dma_start(
                out=out_d[:].rearrange("(b p) -> p b", p=P),
                in_=y2[:],
            )
    nc.compile()
    return nc


def kernel(x, edge_index, W1, b1, W2, b2):
    global LAST_RESULTS
    _install_shims()
    from concourse.bass_utils import run_bass_kernel_spmd

    in_maps, meta = _host_prep(x, edge_index, W1, b1, W2, b2)
    nc = _build_nc(meta["K"], meta["C_idx"], meta["chunks"])
    res = run_bass_kernel_spmd(
        nc, in_maps, core_ids=list(range(NCORES)),
        trace=bool(os.environ.get("BASS_TRACE")),
    )
    LAST_RESULTS = res
    out = np.empty((N, 1), dtype=np.float32)
    for c in range(NCORES):
        yc = res.results[c]["o"]            # [NPAD], rank-ordered
        lo = c * NSLICE
        out[lo + meta["orders"][c], 0] = yc[:NSLICE]
    return out


# revision 13
# speedup vs baseline: 2.9065x; 2.4725x over previous
"""2-layer GCN (GCNConv x2) on 8 Trainium2 NeuronCores via Bass.

Strategy (dst-sharded, dma_gather ELL):
- Nodes sharded into 8 contiguous slices of 31250 (padded to 31360 = 245*128).
- Within each core, nodes are sorted by in-degree and packed into 245 blocks
  of 128 (block b, partition p). Per-block ELL: K_b slot columns per block
  (max in-degree + 1 self slot, maxed across cores).
- Feature table: dis*h per node, bf16, packed 8 nodes per 256B "group" row
  ([31360 groups, 128 bf16] globally). AllGathered per layer (1MB/rank).
- Gather: InstDMAGatherAnt (gpsimd.dma_gather) with int16 group indices,
  8 ELL columns (1024 idxs / 1024 descriptors) per instruction -- descriptor
  ring limit. Index j lands at [p=j%128, col=j//128], so landing position
  encodes the destination slot; no scatter needed.
- Each gathered 256B group holds 8 candidate nodes; a per-slot one-hot bf16
  mask (8 lanes) selects the right node (and zeroes padding slots). One
  strided DVE reduce per block sums over (slots x 8 lanes).
- Self-loops are ELL slots pointing at the node's own group/residue: with
  A_hat = D^-1/2 (A+I) D^-1/2 factored as pre-scale (table rows carry dis*h)
  and post-scale (dis * aggregate), the self term dis^2*h is exact.
"""
import os
import sys
import types

sys.path.insert(0, "/opt/trn_rl_repo")

import numpy as np
import ml_dtypes

N = 250000
E = 4000000
IN_F, HID, OUT = 18, 16, 1
NCORES = 8
NSLICE = N // NCORES            # 31250
BLKS = (NSLICE + 127) // 128    # 245
NPAD = BLKS * 128               # 31360
P = 128
GSEG = NPAD // 8                # 3920 groups per core segment
G_ALL = NCORES * GSEG           # 31360 groups globally (< int16 max)

SCMAX = 64                      # ELL columns per super-chunk (gather tile)
ICOLS = 8                       # ELL columns per dma_gather instruction
NQ = 4                          # SWDGE queues

LAST_RESULTS = None             # test.py reads exec_time_ns from here


def _install_shims():
    """Make run_bass_kernel_spmd(trace=True) work in this container."""
    try:
        import antenv.axon_hooks  # noqa: F401
    except ImportError:
        import antenv
        mod = types.ModuleType("antenv.axon_hooks")
        _hook = [None]
        mod.set_axon_ntff_profile_hook = lambda h: _hook.__setitem__(0, h)
        mod.get_axon_ntff_profile_hook = lambda: _hook[0]
        sys.modules["antenv.axon_hooks"] = mod
        antenv.axon_hooks = mod
        try:
            from trn_agent_boot import trn_boot
            mod.set_axon_ntff_profile_hook(
                trn_boot._ntff_profile_via_ctypes("/opt/axon/libaxon_pjrt.so")
            )
        except Exception:
            pass
    from concourse import bass_utils
    bass_utils.upload_artifacts = lambda tmpdir: tmpdir


def _host_prep(x, edge_index, W1, b1, W2, b2):
    src = np.asarray(edge_index[0], dtype=np.int64).astype(np.int32)
    dst = np.asarray(edge_index[1], dtype=np.int64).astype(np.int32)
    x = np.asarray(x, dtype=np.float32)

    deg_in = np.bincount(dst, minlength=N).astype(np.int64)   # without self loop

    # per-core degree-ascending rank of each node
    rank = np.empty(N, dtype=np.int64)
    orders = []
    for c in range(NCORES):
        lo, hi = c * NSLICE, (c + 1) * NSLICE
        order = np.argsort(deg_in[lo:hi], kind="stable")      # ascending
        orders.append(order)
        rank[lo + order] = np.arange(NSLICE)
    owner = np.arange(N) // NSLICE
    table_row = owner * NPAD + rank                           # global table row

    # common per-block slot counts K_b (max over cores) + 1 self slot
    K = np.zeros(BLKS, dtype=np.int64)
    for c in range(NCORES):
        lo = c * NSLICE
        ds = deg_in[lo + orders[c]]                           # ascending
        ds_pad = np.concatenate([ds, np.zeros(NPAD - NSLICE, np.int64)])
        K = np.maximum(K, ds_pad.reshape(BLKS, P).max(axis=1))
    K = K + 1                                                 # self slot
    off = np.concatenate([[0], np.cumsum(K)]).astype(np.int64)
    C_idx = int(off[-1])

    # place each edge: sorted by dst, k-th in-edge of node d goes to
    # column off[b]+k on partition p, where rank[d] = b*128+p
    es = np.argsort(dst, kind="stable")
    dsts = dst[es]
    srcs = src[es]
    run_first = np.searchsorted(dsts, np.arange(N))           # first pos per node
    k_arr = np.arange(E, dtype=np.int64) - run_first[dsts]
    c_arr = dsts // NSLICE
    r_arr = rank[dsts]
    b_arr = r_arr // P
    p_arr = r_arr % P
    col_arr = off[b_arr] + k_arr
    trow_arr = table_row[srcs]

    # per-slot group index + residue + validity, [NCORES, P, C_idx]
    grp_all = np.zeros((NCORES, P, C_idx), dtype=np.int16)
    res_all = np.zeros((NCORES, P, C_idx), dtype=np.int8)
    val_all = np.zeros((NCORES, P, C_idx), dtype=bool)
    grp_all[c_arr, p_arr, col_arr] = (trow_arr // 8).astype(np.int16)
    res_all[c_arr, p_arr, col_arr] = (trow_arr % 8).astype(np.int8)
    val_all[c_arr, p_arr, col_arr] = True

    # self slots: node (c, b, p) real iff rank < NSLICE; its table row is
    # c*NPAD + b*128 + p; self slot at column off[b] + deg
    for c in range(NCORES):
        lo = c * NSLICE
        deg_ord = np.concatenate(
            [deg_in[lo + orders[c]], np.zeros(NPAD - NSLICE, np.int64)]
        )
        ranks = np.arange(NPAD)
        bs, ps = ranks // P, ranks % P
        cols = off[bs] + deg_ord
        rows = c * NPAD + ranks
        real = ranks < NSLICE
        grp_all[c, ps[real], cols[real]] = (rows[real] // 8).astype(np.int16)
        res_all[c, ps[real], cols[real]] = (rows[real] % 8).astype(np.int8)
        val_all[c, ps[real], cols[real]] = True

    # block-aligned super-chunks of <= SCMAX columns
    chunks = []
    cb0 = 0
    while cb0 < BLKS:
        cb1 = cb0
        while cb1 < BLKS and (off[cb1 + 1] - off[cb0]) <= SCMAX:
            cb1 += 1
        assert cb1 > cb0, f"block {cb0} has K={K[cb0]} > SCMAX={SCMAX}"
        chunks.append((cb0, cb1, int(off[cb0]), int(off[cb1])))
        cb0 = cb1

    # per-core tensors
    in_maps = []
    for c in range(NCORES):
        lo = c * NSLICE
        order = orders[c]
        xT = np.zeros((IN_F, NPAD), dtype=np.float32)
        xT[:, :NSLICE] = x[lo + order].T
        deg_t = np.ones(NPAD, dtype=np.float32)
        deg_t[:NSLICE] = deg_in[lo + order].astype(np.float32) + 1.0
        deg_t = deg_t.reshape(BLKS, P).T.copy()               # [128, 245]

        # idx wrap: element j = s*128+p of any column-aligned instruction is
        # at [j%16, s*8 + p//16]; replicated across the 8 16-partition groups
        idxg = (
            grp_all[c].reshape(8, 16, C_idx).transpose(1, 2, 0).reshape(16, C_idx * 8)
        )
        idxg = np.tile(idxg, (8, 1)).astype(np.int16)

        # mask: [p, s*8 + r] one-hot bf16 (zero for padding slots)
        onehot = (
            np.arange(8, dtype=np.int8)[None, None, :] == res_all[c][:, :, None]
        ) & val_all[c][:, :, None]
        maskt = onehot.reshape(P, C_idx * 8).astype(ml_dtypes.bfloat16)

        in_maps.append({
            "xT": xT,
            "degt": deg_t,
            "idxg": idxg,
            "maskt": maskt,
            "W1": np.asarray(W1, dtype=np.float32),
            "W2r": np.tile(np.asarray(W2, np.float32).reshape(1, HID), (P, 1)),
            "b1r": np.tile(np.asarray(b1, np.float32).reshape(1, HID), (P, 1)),
            "b2r": np.full((P, 1), np.float32(np.asarray(b2).reshape(-1)[0])),
        })
    meta = {"K": K.tolist(), "off": off.tolist(), "C_idx": C_idx,
            "orders": orders, "chunks": chunks}
    return in_maps, meta


def _build_nc(K, C_idx, CHUNKS):
    from concourse import bass, bacc, mybir
    import concourse.tile as tile

    nc = bacc.Bacc("TRN2", target_bir_lowering=False, debug=False,
                   num_devices=NCORES, num_swdge_queues=NQ)
    f32 = mybir.dt.float32
    bf16 = mybir.dt.bfloat16
    i16 = mybir.dt.int16
    xT_d = nc.dram_tensor("xT", [IN_F, NPAD], f32, kind="ExternalInput")
    degt_d = nc.dram_tensor("degt", [P, BLKS], f32, kind="ExternalInput")
    idxg_d = nc.dram_tensor("idxg", [P, C_idx * 8], i16, kind="ExternalInput")
    maskt_d = nc.dram_tensor("maskt", [P, C_idx * 8], bf16, kind="ExternalInput")
    W1_d = nc.dram_tensor("W1", [IN_F, HID], f32, kind="ExternalInput")
    W2r_d = nc.dram_tensor("W2r", [P, HID], f32, kind="ExternalInput")
    b1r_d = nc.dram_tensor("b1r", [P, HID], f32, kind="ExternalInput")
    b2r_d = nc.dram_tensor("b2r", [P, 1], f32, kind="ExternalInput")
    out_d = nc.dram_tensor("o", [NPAD], f32, kind="ExternalOutput")

    FB = BLKS * HID  # 3920 free cols for [p, (b f)] layouts

    OFF = [0]
    for kb in K:
        OFF.append(OFF[-1] + kb)

    with tile.TileContext(nc) as tc:
        with (
            tc.tile_pool(name="const", bufs=1) as cp,
            tc.tile_pool(name="xp", bufs=2) as xp,
            tc.tile_pool(name="ps", bufs=2, space="PSUM") as psp,
            tc.tile_pool(name="wk", bufs=1) as wk,
            tc.tile_pool(name="gth", bufs=3) as gth,
            tc.tile_pool(name="ixp", bufs=3) as ixp,
            tc.tile_pool(name="dram", bufs=1, space="DRAM") as dr,
        ):
            W1s = cp.tile([IN_F, HID], f32)
            nc.sync.dma_start(out=W1s[:], in_=W1_d[:])
            W2s = cp.tile([P, HID], f32)
            nc.sync.dma_start(out=W2s[:], in_=W2r_d[:])
            b1s = cp.tile([P, HID], f32)
            nc.sync.dma_start(out=b1s[:], in_=b1r_d[:])
            b2s = cp.tile([P, 1], f32)
            nc.sync.dma_start(out=b2s[:], in_=b2r_d[:])
            degs = cp.tile([P, BLKS], f32)
            nc.sync.dma_start(out=degs[:], in_=degt_d[:])

            dis = cp.tile([P, BLKS], f32)
            nc.vector.reciprocal(out=dis[:], in_=degs[:])
            nc.scalar.activation(out=dis[:], in_=dis[:],
                                 func=mybir.ActivationFunctionType.Sqrt)

            ag1in = dr.tile([GSEG, 128], bf16)
            table1 = dr.tile([G_ALL, 128], bf16, addr_space="Shared")
            ag2in = dr.tile([GSEG, 128], bf16)
            table2 = dr.tile([G_ALL, 128], bf16, addr_space="Shared")

            disB = dis[:].rearrange("p (b one) -> p b one", one=1)

            # ---- phase A: htab = dis * (x @ W1) in bf16, rank order ----
            htab = wk.tile([P, FB], bf16)
            CHUNK = 32
            for piece in range((BLKS + CHUNK - 1) // CHUNK):
                b0 = piece * CHUNK
                nb = min(CHUNK, BLKS - b0)
                xpc = xp.tile([IN_F, CHUNK * P], f32, tag="xpc")
                nc.sync.dma_start(out=xpc[:, : nb * P],
                                  in_=xT_d[:, b0 * P : (b0 + nb) * P])
                pst = psp.tile([P, CHUNK * HID], f32, tag="pst")
                for j in range(nb):
                    nc.tensor.matmul(
                        out=pst[:, j * HID : (j + 1) * HID],
                        lhsT=xpc[:, j * P : (j + 1) * P],
                        rhs=W1s[:],
                        start=True, stop=True,
                    )
                dis_b = dis[:, b0 : b0 + nb].rearrange("p (b one) -> p b one", one=1)
                nc.vector.tensor_tensor(
                    out=htab[:, b0 * HID : (b0 + nb) * HID],
                    in0=pst[:, : nb * HID].rearrange("p (b f) -> p b f", f=HID),
                    in1=dis_b.to_broadcast([P, nb, HID]),
                    op=mybir.AluOpType.mult,
                )
            nc.sync.dma_start(
                out=ag1in[:].rearrange("(b ph) (pl f) -> (ph pl) b f",
                                       ph=16, pl=8, f=HID),
                in_=htab[:].rearrange("p (b f) -> p b f", f=HID),
            )
            nc.gpsimd.collective_compute(
                "AllGather", mybir.AluOpType.bypass,
                replica_groups=[list(range(NCORES))],
                ins=[ag1in.opt()], outs=[table1.opt()],
            )

            qn = [0]

            def aggregate(table, accname):
                acc = wk.tile([P, FB], f32, name=accname)
                for (b0, b1, s0, s1) in CHUNKS:
                    SC = s1 - s0
                    bt = gth.tile([P, SCMAX * 128], bf16, tag="bt")
                    ixt = ixp.tile([P, SCMAX * 8], i16, tag="ixt")
                    nc.sync.dma_start(out=ixt[:, : SC * 8],
                                      in_=idxg_d[:, s0 * 8 : s1 * 8])
                    mkt = ixp.tile([P, SCMAX * 8], bf16, tag="mkt")
                    nc.sync.dma_start(out=mkt[:, : SC * 8],
                                      in_=maskt_d[:, s0 * 8 : s1 * 8])
                    # gather in <= ICOLS column pieces (descriptor ring limit)
                    for c0 in range(0, SC, ICOLS):
                        c1 = min(c0 + ICOLS, SC)
                        nidx = (c1 - c0) * P
                        nc.gpsimd.dma_gather(
                            out_ap=bt[:, c0 * 128 : c1 * 128].rearrange(
                                "p (s e) -> p s e", e=128
                            ),
                            in_ap=table[:, :],
                            idxs_ap=ixt[:, c0 * 8 : c1 * 8],
                            num_idxs=nidx,
                            num_idxs_reg=nidx,
                            elem_size=128,
                            queue_num=qn[0] % NQ,
                        )
                        qn[0] += 1
                    # select the target node in each gathered 8-group
                    nc.vector.tensor_tensor(
                        out=bt[:, : SC * 128].rearrange(
                            "p (s r f) -> p s r f", r=8, f=HID
                        ),
                        in0=bt[:, : SC * 128].rearrange(
                            "p (s r f) -> p s r f", r=8, f=HID
                        ),
                        in1=mkt[:, : SC * 8].rearrange(
                            "p (s r one) -> p s r one", r=8, one=1
                        ).to_broadcast([P, SC, 8, HID]),
                        op=mybir.AluOpType.mult,
                    )
                    for b in range(b0, b1):
                        kb = K[b]
                        o = OFF[b] - s0
                        nc.vector.reduce_sum(
                            out=acc[:, b * HID : (b + 1) * HID],
                            in_=bt[:, o * 128 : (o + kb) * 128].rearrange(
                                "p (kr f) -> p f kr", f=HID
                            ),
                            axis=mybir.AxisListType.X,
                        )
                return acc

            # ---- layer 1 (self term included via ELL self slots) ----
            acc1 = aggregate(table1, "acc1")
            # y1 = acc1 * dis ; h1 = relu(y1 + b1) ; htab2 = dis * h1
            nc.vector.tensor_tensor(
                out=acc1[:],
                in0=acc1[:].rearrange("p (b f) -> p b f", f=HID),
                in1=disB.to_broadcast([P, BLKS, HID]),
                op=mybir.AluOpType.mult,
            )
            nc.vector.tensor_tensor(
                out=acc1[:],
                in0=acc1[:].rearrange("p (b f) -> p b f", f=HID),
                in1=b1s[:].rearrange("p (one f) -> p one f", one=1).to_broadcast([P, BLKS, HID]),
                op=mybir.AluOpType.add,
            )
            nc.scalar.activation(out=acc1[:], in_=acc1[:],
                                 func=mybir.ActivationFunctionType.Relu)
            htab2 = wk.tile([P, FB], bf16)
            nc.vector.tensor_tensor(
                out=htab2[:],
                in0=acc1[:].rearrange("p (b f) -> p b f", f=HID),
                in1=disB.to_broadcast([P, BLKS, HID]),
                op=mybir.AluOpType.mult,
            )

            nc.sync.dma_start(
                out=ag2in[:].rearrange("(b ph) (pl f) -> (ph pl) b f",
                                       ph=16, pl=8, f=HID),
                in_=htab2[:].rearrange("p (b f) -> p b f", f=HID),
            )
            nc.gpsimd.collective_compute(
                "AllGather", mybir.AluOpType.bypass,
                replica_groups=[list(range(NCORES))],
                ins=[ag2in.opt()], outs=[table2.opt()],
            )

            # ---- layer 2 ----
            acc2 = aggregate(table2, "acc2")
            nc.vector.tensor_tensor(
                out=acc2[:],
                in0=acc2[:].rearrange("p (b f) -> p b f", f=HID),
                in1=disB.to_broadcast([P, BLKS, HID]),
                op=mybir.AluOpType.mult,
            )
            nc.vector.tensor_tensor(
                out=acc2[:],
                in0=acc2[:].rearrange("p (b f) -> p b f", f=HID),
                in1=W2s[:].rearrange("p (one f) -> p one f", one=1).to_broadcast([P, BLKS, HID]),
                op=mybir.AluOpType.mult,
            )
            y2 = wk.tile([P, BLKS], f32)
            nc.vector.reduce_sum(
                out=y2[:],
                in_=acc2[:].rearrange("p (b f) -> p b f", f=HID),
                axis=mybir.AxisListType.X,
            )
            nc.vector.tensor_tensor(
                out=y2[:],
                in0=y2[:],
                in1=b2s[:].to_broadcast([P, BLKS]),
                op=mybir.AluOpType.add,
            )
            nc.sync.dma_start(
                out=out_d[:].rearrange("(b p) -> p b", p=P),
                in_=y2[:],
            )
    nc.compile()
    return nc


def kernel(x, edge_index, W1, b1, W2, b2):
    global LAST_RESULTS
    _install_shims()
    from concourse.bass_utils import run_bass_kernel_spmd

    in_maps, meta = _host_prep(x, edge_index, W1, b1, W2, b2)
    nc = _build_nc(meta["K"], meta["C_idx"], meta["chunks"])
    res = run_bass_kernel_spmd(
        nc, in_maps, core_ids=list(range(NCORES)),
        trace=bool(os.environ.get("BASS_TRACE")),
    )
    LAST_RESULTS = res
    out = np.empty((N, 1), dtype=np.float32)
    for c in range(NCORES):
        yc = res.results[c]["o"]            # [NPAD], rank-ordered
        lo = c * NSLICE
        out[lo + meta["orders"][c], 0] = yc[:NSLICE]
    return out


# revision 14
# speedup vs baseline: 2.9498x; 1.0149x over previous
"""2-layer GCN (GCNConv x2) on 8 Trainium2 NeuronCores via Bass.

Strategy (dst-sharded, dma_gather ELL):
- Nodes sharded into 8 contiguous slices of 31250 (padded to 31360 = 245*128).
- Within each core, nodes are sorted by in-degree and packed into 245 blocks
  of 128 (block b, partition p). Per-block ELL: K_b slot columns per block
  (max in-degree + 1 self slot, maxed across cores).
- Feature table: dis*h per node, bf16, packed 8 nodes per 256B "group" row
  ([31360 groups, 128 bf16] globally). AllGathered per layer (1MB/rank).
- Gather: InstDMAGatherAnt (gpsimd.dma_gather) with int16 group indices,
  8 ELL columns (1024 idxs / 1024 descriptors) per instruction -- descriptor
  ring limit. Index j lands at [p=j%128, col=j//128], so landing position
  encodes the destination slot; no scatter needed.
- Each gathered 256B group holds 8 candidate nodes; a per-slot one-hot bf16
  mask (8 lanes) selects the right node (and zeroes padding slots). One
  strided DVE reduce per block sums over (slots x 8 lanes).
- Self-loops are ELL slots pointing at the node's own group/residue: with
  A_hat = D^-1/2 (A+I) D^-1/2 factored as pre-scale (table rows carry dis*h)
  and post-scale (dis * aggregate), the self term dis^2*h is exact.
"""
import os
import sys
import types

sys.path.insert(0, "/opt/trn_rl_repo")

import numpy as np
import ml_dtypes

N = 250000
E = 4000000
IN_F, HID, OUT = 18, 16, 1
NCORES = 8
NSLICE = N // NCORES            # 31250
BLKS = (NSLICE + 127) // 128    # 245
NPAD = BLKS * 128               # 31360
P = 128
GSEG = NPAD // 8                # 3920 groups per core segment
G_ALL = NCORES * GSEG           # 31360 groups globally (< int16 max)

SCMAX = 64                      # ELL columns per super-chunk (gather tile)
ICOLS = 8                       # ELL columns per dma_gather instruction
NQ = 4                          # SWDGE queues

LAST_RESULTS = None             # test.py reads exec_time_ns from here


def _install_shims():
    """Make run_bass_kernel_spmd(trace=True) work in this container."""
    try:
        import antenv.axon_hooks  # noqa: F401
    except ImportError:
        import antenv
        mod = types.ModuleType("antenv.axon_hooks")
        _hook = [None]
        mod.set_axon_ntff_profile_hook = lambda h: _hook.__setitem__(0, h)
        mod.get_axon_ntff_profile_hook = lambda: _hook[0]
        sys.modules["antenv.axon_hooks"] = mod
        antenv.axon_hooks = mod
        try:
            from trn_agent_boot import trn_boot
            mod.set_axon_ntff_profile_hook(
                trn_boot._ntff_profile_via_ctypes("/opt/axon/libaxon_pjrt.so")
            )
        except Exception:
            pass
    from concourse import bass_utils
    bass_utils.upload_artifacts = lambda tmpdir: tmpdir


def _host_prep(x, edge_index, W1, b1, W2, b2):
    src = np.asarray(edge_index[0], dtype=np.int64).astype(np.int32)
    dst = np.asarray(edge_index[1], dtype=np.int64).astype(np.int32)
    x = np.asarray(x, dtype=np.float32)

    deg_in = np.bincount(dst, minlength=N).astype(np.int64)   # without self loop

    # per-core degree-ascending rank of each node
    rank = np.empty(N, dtype=np.int64)
    orders = []
    for c in range(NCORES):
        lo, hi = c * NSLICE, (c + 1) * NSLICE
        order = np.argsort(deg_in[lo:hi], kind="stable")      # ascending
        orders.append(order)
        rank[lo + order] = np.arange(NSLICE)
    owner = np.arange(N) // NSLICE
    table_row = owner * NPAD + rank                           # global table row

    # common per-block slot counts K_b (max over cores) + 1 self slot
    K = np.zeros(BLKS, dtype=np.int64)
    for c in range(NCORES):
        lo = c * NSLICE
        ds = deg_in[lo + orders[c]]                           # ascending
        ds_pad = np.concatenate([ds, np.zeros(NPAD - NSLICE, np.int64)])
        K = np.maximum(K, ds_pad.reshape(BLKS, P).max(axis=1))
    K = K + 1                                                 # self slot
    off = np.concatenate([[0], np.cumsum(K)]).astype(np.int64)
    C_idx = int(off[-1])

    # place each edge: sorted by dst, k-th in-edge of node d goes to
    # column off[b]+k on partition p, where rank[d] = b*128+p
    es = np.argsort(dst, kind="stable")
    dsts = dst[es]
    srcs = src[es]
    run_first = np.searchsorted(dsts, np.arange(N))           # first pos per node
    k_arr = np.arange(E, dtype=np.int64) - run_first[dsts]
    c_arr = dsts // NSLICE
    r_arr = rank[dsts]
    b_arr = r_arr // P
    p_arr = r_arr % P
    col_arr = off[b_arr] + k_arr
    trow_arr = table_row[srcs]

    # per-slot group index + residue + validity, [NCORES, P, C_idx]
    grp_all = np.zeros((NCORES, P, C_idx), dtype=np.int16)
    res_all = np.zeros((NCORES, P, C_idx), dtype=np.int8)
    val_all = np.zeros((NCORES, P, C_idx), dtype=bool)
    grp_all[c_arr, p_arr, col_arr] = (trow_arr // 8).astype(np.int16)
    res_all[c_arr, p_arr, col_arr] = (trow_arr % 8).astype(np.int8)
    val_all[c_arr, p_arr, col_arr] = True

    # self slots: node (c, b, p) real iff rank < NSLICE; its table row is
    # c*NPAD + b*128 + p; self slot at column off[b] + deg
    for c in range(NCORES):
        lo = c * NSLICE
        deg_ord = np.concatenate(
            [deg_in[lo + orders[c]], np.zeros(NPAD - NSLICE, np.int64)]
        )
        ranks = np.arange(NPAD)
        bs, ps = ranks // P, ranks % P
        cols = off[bs] + deg_ord
        rows = c * NPAD + ranks
        real = ranks < NSLICE
        grp_all[c, ps[real], cols[real]] = (rows[real] // 8).astype(np.int16)
        res_all[c, ps[real], cols[real]] = (rows[real] % 8).astype(np.int8)
        val_all[c, ps[real], cols[real]] = True

    # block-aligned super-chunks of <= SCMAX columns
    chunks = []
    cb0 = 0
    while cb0 < BLKS:
        cb1 = cb0
        while cb1 < BLKS and (off[cb1 + 1] - off[cb0]) <= SCMAX:
            cb1 += 1
        assert cb1 > cb0, f"block {cb0} has K={K[cb0]} > SCMAX={SCMAX}"
        chunks.append((cb0, cb1, int(off[cb0]), int(off[cb1])))
        cb0 = cb1

    # per-core tensors
    in_maps = []
    for c in range(NCORES):
        lo = c * NSLICE
        order = orders[c]
        xT = np.zeros((IN_F, NPAD), dtype=np.float32)
        xT[:, :NSLICE] = x[lo + order].T
        deg_t = np.ones(NPAD, dtype=np.float32)
        deg_t[:NSLICE] = deg_in[lo + order].astype(np.float32) + 1.0
        deg_t = deg_t.reshape(BLKS, P).T.copy()               # [128, 245]

        # idx wrap: element j = s*128+p of any column-aligned instruction is
        # at [j%16, s*8 + p//16]; replicated across the 8 16-partition groups
        idxg = (
            grp_all[c].reshape(8, 16, C_idx).transpose(1, 2, 0).reshape(16, C_idx * 8)
        )
        idxg = np.tile(idxg, (8, 1)).astype(np.int16)

        # mask: [p, s*8 + r] one-hot bf16 (zero for padding slots)
        onehot = (
            np.arange(8, dtype=np.int8)[None, None, :] == res_all[c][:, :, None]
        ) & val_all[c][:, :, None]
        maskt = onehot.reshape(P, C_idx * 8).astype(ml_dtypes.bfloat16)

        in_maps.append({
            "xT": xT,
            "degt": deg_t,
            "idxg": idxg,
            "maskt": maskt,
            "W1": np.asarray(W1, dtype=np.float32),
            "W2r": np.tile(np.asarray(W2, np.float32).reshape(1, HID), (P, 1)),
            "b1r": np.tile(np.asarray(b1, np.float32).reshape(1, HID), (P, 1)),
            "b2r": np.full((P, 1), np.float32(np.asarray(b2).reshape(-1)[0])),
        })
    meta = {"K": K.tolist(), "off": off.tolist(), "C_idx": C_idx,
            "orders": orders, "chunks": chunks}
    return in_maps, meta


def _build_nc(K, C_idx, CHUNKS):
    from concourse import bass, bacc, mybir
    import concourse.tile as tile

    nc = bacc.Bacc("TRN2", target_bir_lowering=False, debug=False,
                   num_devices=NCORES, num_swdge_queues=NQ)
    f32 = mybir.dt.float32
    bf16 = mybir.dt.bfloat16
    i16 = mybir.dt.int16
    xT_d = nc.dram_tensor("xT", [IN_F, NPAD], f32, kind="ExternalInput")
    degt_d = nc.dram_tensor("degt", [P, BLKS], f32, kind="ExternalInput")
    idxg_d = nc.dram_tensor("idxg", [P, C_idx * 8], i16, kind="ExternalInput")
    maskt_d = nc.dram_tensor("maskt", [P, C_idx * 8], bf16, kind="ExternalInput")
    W1_d = nc.dram_tensor("W1", [IN_F, HID], f32, kind="ExternalInput")
    W2r_d = nc.dram_tensor("W2r", [P, HID], f32, kind="ExternalInput")
    b1r_d = nc.dram_tensor("b1r", [P, HID], f32, kind="ExternalInput")
    b2r_d = nc.dram_tensor("b2r", [P, 1], f32, kind="ExternalInput")
    out_d = nc.dram_tensor("o", [NPAD], f32, kind="ExternalOutput")

    FB = BLKS * HID  # 3920 free cols for [p, (b f)] layouts

    OFF = [0]
    for kb in K:
        OFF.append(OFF[-1] + kb)

    with tile.TileContext(nc) as tc:
        with (
            tc.tile_pool(name="const", bufs=1) as cp,
            tc.tile_pool(name="xp", bufs=2) as xp,
            tc.tile_pool(name="ps", bufs=2, space="PSUM") as psp,
            tc.tile_pool(name="wk", bufs=1) as wk,
            tc.tile_pool(name="gth", bufs=3) as gth,
            tc.tile_pool(name="ixp", bufs=3) as ixp,
            tc.tile_pool(name="dram", bufs=1, space="DRAM") as dr,
        ):
            W1s = cp.tile([IN_F, HID], f32)
            nc.sync.dma_start(out=W1s[:], in_=W1_d[:])
            W2s = cp.tile([P, HID], f32)
            nc.sync.dma_start(out=W2s[:], in_=W2r_d[:])
            b1s = cp.tile([P, HID], f32)
            nc.sync.dma_start(out=b1s[:], in_=b1r_d[:])
            b2s = cp.tile([P, 1], f32)
            nc.sync.dma_start(out=b2s[:], in_=b2r_d[:])
            degs = cp.tile([P, BLKS], f32)
            nc.sync.dma_start(out=degs[:], in_=degt_d[:])

            dis = cp.tile([P, BLKS], f32)
            nc.vector.reciprocal(out=dis[:], in_=degs[:])
            nc.scalar.activation(out=dis[:], in_=dis[:],
                                 func=mybir.ActivationFunctionType.Sqrt)

            ag1in = dr.tile([GSEG, 128], bf16)
            table1 = dr.tile([G_ALL, 128], bf16, addr_space="Shared")
            ag2in = dr.tile([GSEG, 128], bf16)
            table2 = dr.tile([G_ALL, 128], bf16, addr_space="Shared")

            disB = dis[:].rearrange("p (b one) -> p b one", one=1)

            # ---- phase A: htab = dis * (x @ W1) in bf16, rank order ----
            htab = wk.tile([P, FB], bf16)
            CHUNK = 32
            for piece in range((BLKS + CHUNK - 1) // CHUNK):
                b0 = piece * CHUNK
                nb = min(CHUNK, BLKS - b0)
                xpc = xp.tile([IN_F, CHUNK * P], f32, tag="xpc")
                nc.sync.dma_start(out=xpc[:, : nb * P],
                                  in_=xT_d[:, b0 * P : (b0 + nb) * P])
                pst = psp.tile([P, CHUNK * HID], f32, tag="pst")
                for j in range(nb):
                    nc.tensor.matmul(
                        out=pst[:, j * HID : (j + 1) * HID],
                        lhsT=xpc[:, j * P : (j + 1) * P],
                        rhs=W1s[:],
                        start=True, stop=True,
                    )
                dis_b = dis[:, b0 : b0 + nb].rearrange("p (b one) -> p b one", one=1)
                nc.vector.tensor_tensor(
                    out=htab[:, b0 * HID : (b0 + nb) * HID],
                    in0=pst[:, : nb * HID].rearrange("p (b f) -> p b f", f=HID),
                    in1=dis_b.to_broadcast([P, nb, HID]),
                    op=mybir.AluOpType.mult,
                )
            nc.sync.dma_start(
                out=ag1in[:].rearrange("(b ph) (pl f) -> (ph pl) b f",
                                       ph=16, pl=8, f=HID),
                in_=htab[:].rearrange("p (b f) -> p b f", f=HID),
            )
            nc.gpsimd.collective_compute(
                "AllGather", mybir.AluOpType.bypass,
                replica_groups=[list(range(NCORES))],
                ins=[ag1in.opt()], outs=[table1.opt()],
            )

            qn = [0]

            def aggregate(table, accname):
                acc = wk.tile([P, FB], f32, name=accname)
                for (b0, b1, s0, s1) in CHUNKS:
                    SC = s1 - s0
                    bt = gth.tile([P, SCMAX * 128], bf16, tag="bt")
                    ixt = ixp.tile([P, SCMAX * 8], i16, tag="ixt")
                    nc.sync.dma_start(out=ixt[:, : SC * 8],
                                      in_=idxg_d[:, s0 * 8 : s1 * 8])
                    mkt = ixp.tile([P, SCMAX * 8], bf16, tag="mkt")
                    nc.sync.dma_start(out=mkt[:, : SC * 8],
                                      in_=maskt_d[:, s0 * 8 : s1 * 8])
                    # gather in <= ICOLS column pieces (descriptor ring limit)
                    for c0 in range(0, SC, ICOLS):
                        c1 = min(c0 + ICOLS, SC)
                        nidx = (c1 - c0) * P
                        nc.gpsimd.dma_gather(
                            out_ap=bt[:, c0 * 128 : c1 * 128].rearrange(
                                "p (s e) -> p s e", e=128
                            ),
                            in_ap=table[:, :],
                            idxs_ap=ixt[:, c0 * 8 : c1 * 8],
                            num_idxs=nidx,
                            num_idxs_reg=nidx,
                            elem_size=128,
                            queue_num=qn[0] % NQ,
                        )
                        qn[0] += 1
                    # select the target node in each gathered 8-group
                    nc.vector.tensor_tensor(
                        out=bt[:, : SC * 128].rearrange(
                            "p (sr f) -> p sr f", f=HID
                        ),
                        in0=bt[:, : SC * 128].rearrange(
                            "p (sr f) -> p sr f", f=HID
                        ),
                        in1=mkt[:, : SC * 8].rearrange(
                            "p (sr one) -> p sr one", one=1
                        ).to_broadcast([P, SC * 8, HID]),
                        op=mybir.AluOpType.mult,
                    )
                    # in-place tree-sum over the 8 lanes: each slot has at
                    # most one nonzero lane, so bf16 sums are exact; the
                    # selected value ends at lane 0 (first 16 cols per slot)
                    for h, e in ((2, 64), (4, 32), (8, 16)):
                        v = bt[:, : SC * 128].rearrange(
                            "p (s h e) -> p s h e", h=h, e=e
                        )
                        nc.vector.tensor_tensor(
                            out=v[:, :, 0:1, :], in0=v[:, :, 0:1, :],
                            in1=v[:, :, 1:2, :], op=mybir.AluOpType.add,
                        )
                    # per-block sum over slot columns (batch equal-K runs)
                    b = b0
                    while b < b1:
                        be = b
                        while be < b1 and K[be] == K[b]:
                            be += 1
                        kb = K[b]
                        nb = be - b
                        o = OFF[b] - s0
                        nc.vector.reduce_sum(
                            out=acc[:, b * HID : be * HID].rearrange(
                                "p (nb f) -> p nb f", f=HID
                            ),
                            in_=bt[:, o * 128 : (o + nb * kb) * 128].rearrange(
                                "p (nb k e) -> p nb e k", k=kb, e=128
                            )[:, :, :HID, :],
                            axis=mybir.AxisListType.X,
                        )
                        b = be
                return acc

            # ---- layer 1 (self term included via ELL self slots) ----
            acc1 = aggregate(table1, "acc1")
            # y1 = acc1 * dis ; h1 = relu(y1 + b1) ; htab2 = dis * h1
            nc.vector.tensor_tensor(
                out=acc1[:],
                in0=acc1[:].rearrange("p (b f) -> p b f", f=HID),
                in1=disB.to_broadcast([P, BLKS, HID]),
                op=mybir.AluOpType.mult,
            )
            nc.vector.tensor_tensor(
                out=acc1[:],
                in0=acc1[:].rearrange("p (b f) -> p b f", f=HID),
                in1=b1s[:].rearrange("p (one f) -> p one f", one=1).to_broadcast([P, BLKS, HID]),
                op=mybir.AluOpType.add,
            )
            nc.scalar.activation(out=acc1[:], in_=acc1[:],
                                 func=mybir.ActivationFunctionType.Relu)
            htab2 = wk.tile([P, FB], bf16)
            nc.vector.tensor_tensor(
                out=htab2[:],
                in0=acc1[:].rearrange("p (b f) -> p b f", f=HID),
                in1=disB.to_broadcast([P, BLKS, HID]),
                op=mybir.AluOpType.mult,
            )

            nc.sync.dma_start(
                out=ag2in[:].rearrange("(b ph) (pl f) -> (ph pl) b f",
                                       ph=16, pl=8, f=HID),
                in_=htab2[:].rearrange("p (b f) -> p b f", f=HID),
            )
            nc.gpsimd.collective_compute(
                "AllGather", mybir.AluOpType.bypass,
                replica_groups=[list(range(NCORES))],
                ins=[ag2in.opt()], outs=[table2.opt()],
            )

            # ---- layer 2 ----
            acc2 = aggregate(table2, "acc2")
            nc.vector.tensor_tensor(
                out=acc2[:],
                in0=acc2[:].rearrange("p (b f) -> p b f", f=HID),
                in1=disB.to_broadcast([P, BLKS, HID]),
                op=mybir.AluOpType.mult,
            )
            nc.vector.tensor_tensor(
                out=acc2[:],
                in0=acc2[:].rearrange("p (b f) -> p b f", f=HID),
                in1=W2s[:].rearrange("p (one f) -> p one f", one=1).to_broadcast([P, BLKS, HID]),
                op=mybir.AluOpType.mult,
            )
            y2 = wk.tile([P, BLKS], f32)
            nc.vector.reduce_sum(
                out=y2[:],
                in_=acc2[:].rearrange("p (b f) -> p b f", f=HID),
                axis=mybir.AxisListType.X,
            )
            nc.vector.tensor_tensor(
                out=y2[:],
                in0=y2[:],
                in1=b2s[:].to_broadcast([P, BLKS]),
                op=mybir.AluOpType.add,
            )
            nc.sync.dma_start(
                out=out_d[:].rearrange("(b p) -> p b", p=P),
                in_=y2[:],
            )
    nc.compile()
    return nc


def kernel(x, edge_index, W1, b1, W2, b2):
    global LAST_RESULTS
    _install_shims()
    from concourse.bass_utils import run_bass_kernel_spmd

    in_maps, meta = _host_prep(x, edge_index, W1, b1, W2, b2)
    nc = _build_nc(meta["K"], meta["C_idx"], meta["chunks"])
    res = run_bass_kernel_spmd(
        nc, in_maps, core_ids=list(range(NCORES)),
        trace=bool(os.environ.get("BASS_TRACE")),
    )
    LAST_RESULTS = res
    out = np.empty((N, 1), dtype=np.float32)
    for c in range(NCORES):
        yc = res.results[c]["o"]            # [NPAD], rank-ordered
        lo = c * NSLICE
        out[lo + meta["orders"][c], 0] = yc[:NSLICE]
    return out


# revision 20
# speedup vs baseline: 3.2946x; 1.1169x over previous
"""2-layer GCN (GCNConv x2) on 8 Trainium2 NeuronCores via Bass.

Strategy (dst-sharded, dma_gather ELL):
- Nodes sharded into 8 contiguous slices of 31250 (padded to 31360 = 245*128).
- Within each core, nodes are sorted by in-degree and packed into 245 blocks
  of 128 (block b, partition p). Per-block ELL: K_b slot columns per block
  (max in-degree + 1 self slot, maxed across cores).
- Feature table: dis*h per node, bf16, packed 8 nodes per 256B "group" row
  ([31360 groups, 128 bf16] globally). AllGathered per layer (1MB/rank).
- Gather: InstDMAGatherAnt (gpsimd.dma_gather) with int16 group indices,
  8 ELL columns (1024 idxs / 1024 descriptors) per instruction -- descriptor
  ring limit. Index j lands at [p=j%128, col=j//128], so landing position
  encodes the destination slot; no scatter needed.
- Each gathered 256B group holds 8 candidate nodes; a per-slot one-hot bf16
  mask (8 lanes) selects the right node (and zeroes padding slots). One
  strided DVE reduce per block sums over (slots x 8 lanes).
- Self-loops are ELL slots pointing at the node's own group/residue: with
  A_hat = D^-1/2 (A+I) D^-1/2 factored as pre-scale (table rows carry dis*h)
  and post-scale (dis * aggregate), the self term dis^2*h is exact.
"""
import os
import sys
import types

sys.path.insert(0, "/opt/trn_rl_repo")

import numpy as np
import ml_dtypes

N = 250000
E = 4000000
IN_F, HID, OUT = 18, 16, 1
NCORES = 8
NSLICE = N // NCORES            # 31250
BLKS = (NSLICE + 127) // 128    # 245
NPAD = BLKS * 128               # 31360
P = 128
GSEG = NPAD // 8                # 3920 groups per core segment
G_ALL = NCORES * GSEG           # 31360 groups globally (< int16 max)

SCMAX = 64                      # ELL columns per super-chunk (gather tile)
ICOLS = 8                       # ELL columns per dma_gather instruction
NQ = 4                          # SWDGE queues

LAST_RESULTS = None             # test.py reads exec_time_ns from here


def _install_shims():
    """Make run_bass_kernel_spmd(trace=True) work in this container."""
    try:
        import antenv.axon_hooks  # noqa: F401
    except ImportError:
        import antenv
        mod = types.ModuleType("antenv.axon_hooks")
        _hook = [None]
        mod.set_axon_ntff_profile_hook = lambda h: _hook.__setitem__(0, h)
        mod.get_axon_ntff_profile_hook = lambda: _hook[0]
        sys.modules["antenv.axon_hooks"] = mod
        antenv.axon_hooks = mod
        try:
            from trn_agent_boot import trn_boot
            mod.set_axon_ntff_profile_hook(
                trn_boot._ntff_profile_via_ctypes("/opt/axon/libaxon_pjrt.so")
            )
        except Exception:
            pass
    from concourse import bass_utils
    bass_utils.upload_artifacts = lambda tmpdir: tmpdir


def _host_prep(x, edge_index, W1, b1, W2, b2):
    src = np.asarray(edge_index[0], dtype=np.int64).astype(np.int32)
    dst = np.asarray(edge_index[1], dtype=np.int64).astype(np.int32)
    x = np.asarray(x, dtype=np.float32)

    deg_in = np.bincount(dst, minlength=N).astype(np.int64)   # without self loop

    # per-core degree-ascending rank of each node
    rank = np.empty(N, dtype=np.int64)
    orders = []
    for c in range(NCORES):
        lo, hi = c * NSLICE, (c + 1) * NSLICE
        order = np.argsort(deg_in[lo:hi], kind="stable")      # ascending
        orders.append(order)
        rank[lo + order] = np.arange(NSLICE)
    owner = np.arange(N) // NSLICE
    table_row = owner * NPAD + rank                           # global table row

    # common per-block slot counts K_b (max over cores) + 1 self slot
    K = np.zeros(BLKS, dtype=np.int64)
    for c in range(NCORES):
        lo = c * NSLICE
        ds = deg_in[lo + orders[c]]                           # ascending
        ds_pad = np.concatenate([ds, np.zeros(NPAD - NSLICE, np.int64)])
        K = np.maximum(K, ds_pad.reshape(BLKS, P).max(axis=1))
    K = np.maximum(K, 1)
    off = np.concatenate([[0], np.cumsum(K)]).astype(np.int64)
    C_idx = int(off[-1])

    # place each edge: sorted by dst, k-th in-edge of node d goes to
    # column off[b]+k on partition p, where rank[d] = b*128+p
    es = np.argsort(dst, kind="stable")
    dsts = dst[es]
    srcs = src[es]
    run_first = np.searchsorted(dsts, np.arange(N))           # first pos per node
    k_arr = np.arange(E, dtype=np.int64) - run_first[dsts]
    c_arr = dsts // NSLICE
    r_arr = rank[dsts]
    b_arr = r_arr // P
    p_arr = r_arr % P
    col_arr = off[b_arr] + k_arr
    trow_arr = table_row[srcs]

    # per-slot group index + residue + validity, [NCORES, P, C_idx]
    grp_all = np.zeros((NCORES, P, C_idx), dtype=np.int16)
    res_all = np.zeros((NCORES, P, C_idx), dtype=np.int8)
    val_all = np.zeros((NCORES, P, C_idx), dtype=bool)
    grp_all[c_arr, p_arr, col_arr] = (trow_arr // 8).astype(np.int16)
    res_all[c_arr, p_arr, col_arr] = (trow_arr % 8).astype(np.int8)
    val_all[c_arr, p_arr, col_arr] = True

    # block-aligned super-chunks of <= SCMAX columns
    chunks = []
    cb0 = 0
    while cb0 < BLKS:
        cb1 = cb0
        while cb1 < BLKS and (off[cb1 + 1] - off[cb0]) <= SCMAX:
            cb1 += 1
        assert cb1 > cb0, f"block {cb0} has K={K[cb0]} > SCMAX={SCMAX}"
        chunks.append((cb0, cb1, int(off[cb0]), int(off[cb1])))
        cb0 = cb1

    # per-core tensors
    in_maps = []
    for c in range(NCORES):
        lo = c * NSLICE
        order = orders[c]
        xT = np.zeros((IN_F, NPAD), dtype=np.float32)
        xT[:, :NSLICE] = x[lo + order].T
        deg_t = np.ones(NPAD, dtype=np.float32)
        deg_t[:NSLICE] = deg_in[lo + order].astype(np.float32) + 1.0
        deg_t = deg_t.reshape(BLKS, P).T.copy()               # [128, 245]

        # idx wrap: element j = s*128+p of any column-aligned instruction is
        # at [j%16, s*8 + p//16]; replicated across the 8 16-partition groups
        idxg = (
            grp_all[c].reshape(8, 16, C_idx).transpose(1, 2, 0).reshape(16, C_idx * 8)
        )
        idxg = np.tile(idxg, (8, 1)).astype(np.int16)

        # mask: [p, s*8 + r] one-hot bf16 (zero for padding slots)
        onehot = (
            np.arange(8, dtype=np.int8)[None, None, :] == res_all[c][:, :, None]
        ) & val_all[c][:, :, None]
        maskt = onehot.reshape(P, C_idx * 8).astype(ml_dtypes.bfloat16)

        in_maps.append({
            "xT": xT,
            "degt": deg_t,
            "idxg": idxg,
            "maskt": maskt,
            "W1": np.asarray(W1, dtype=np.float32),
            "W2r": np.tile(np.asarray(W2, np.float32).reshape(1, HID), (P, 1)),
            "b1r": np.tile(np.asarray(b1, np.float32).reshape(1, HID), (P, 1)),
            "b2r": np.full((P, 1), np.float32(np.asarray(b2).reshape(-1)[0])),
        })
    meta = {"K": K.tolist(), "off": off.tolist(), "C_idx": C_idx,
            "orders": orders, "chunks": chunks}
    return in_maps, meta


def _build_nc(K, C_idx, CHUNKS):
    from concourse import bass, bacc, mybir
    import concourse.tile as tile

    nc = bacc.Bacc("TRN2", target_bir_lowering=False, debug=False,
                   num_devices=NCORES, num_swdge_queues=NQ)
    f32 = mybir.dt.float32
    bf16 = mybir.dt.bfloat16
    i16 = mybir.dt.int16
    xT_d = nc.dram_tensor("xT", [IN_F, NPAD], f32, kind="ExternalInput")
    degt_d = nc.dram_tensor("degt", [P, BLKS], f32, kind="ExternalInput")
    idxg_d = nc.dram_tensor("idxg", [P, C_idx * 8], i16, kind="ExternalInput")
    maskt_d = nc.dram_tensor("maskt", [P, C_idx * 8], bf16, kind="ExternalInput")
    W1_d = nc.dram_tensor("W1", [IN_F, HID], f32, kind="ExternalInput")
    W2r_d = nc.dram_tensor("W2r", [P, HID], f32, kind="ExternalInput")
    b1r_d = nc.dram_tensor("b1r", [P, HID], f32, kind="ExternalInput")
    b2r_d = nc.dram_tensor("b2r", [P, 1], f32, kind="ExternalInput")
    out_d = nc.dram_tensor("o", [NPAD], f32, kind="ExternalOutput")

    FB = BLKS * HID  # 3920 free cols for [p, (b f)] layouts

    OFF = [0]
    for kb in K:
        OFF.append(OFF[-1] + kb)

    with tile.TileContext(nc) as tc:
        with (
            tc.tile_pool(name="const", bufs=1) as cp,
            tc.tile_pool(name="xp", bufs=2) as xp,
            tc.tile_pool(name="ps", bufs=2, space="PSUM") as psp,
            tc.tile_pool(name="wk", bufs=1) as wk,
            tc.tile_pool(name="gth", bufs=5) as gth,
            tc.tile_pool(name="ixp", bufs=5) as ixp,
            tc.tile_pool(name="dram", bufs=1, space="DRAM") as dr,
        ):
            W1s = cp.tile([IN_F, HID], f32)
            nc.sync.dma_start(out=W1s[:], in_=W1_d[:])
            W2s = cp.tile([P, HID], f32)
            nc.sync.dma_start(out=W2s[:], in_=W2r_d[:])
            b1s = cp.tile([P, HID], f32)
            nc.sync.dma_start(out=b1s[:], in_=b1r_d[:])
            b2s = cp.tile([P, 1], f32)
            nc.sync.dma_start(out=b2s[:], in_=b2r_d[:])
            degs = cp.tile([P, BLKS], f32)
            nc.sync.dma_start(out=degs[:], in_=degt_d[:])

            dis = cp.tile([P, BLKS], f32)
            nc.vector.reciprocal(out=dis[:], in_=degs[:])
            nc.scalar.activation(out=dis[:], in_=dis[:],
                                 func=mybir.ActivationFunctionType.Sqrt)

            ag1in = dr.tile([GSEG, 128], bf16)
            table1 = dr.tile([G_ALL, 128], bf16, addr_space="Shared")
            ag2in = dr.tile([GSEG, 128], bf16)
            table2 = dr.tile([G_ALL, 128], bf16, addr_space="Shared")

            disB = dis[:].rearrange("p (b one) -> p b one", one=1)

            # ---- phase A: htab = dis * (x @ W1) in bf16, rank order ----
            htab = wk.tile([P, FB], bf16)
            CHUNK = 16
            for piece in range((BLKS + CHUNK - 1) // CHUNK):
                b0 = piece * CHUNK
                nb = min(CHUNK, BLKS - b0)
                xpc = xp.tile([IN_F, CHUNK * P], f32, tag="xpc")
                nc.sync.dma_start(out=xpc[:, : nb * P],
                                  in_=xT_d[:, b0 * P : (b0 + nb) * P])
                pst = psp.tile([P, CHUNK * HID], f32, tag="pst")
                for j in range(nb):
                    nc.tensor.matmul(
                        out=pst[:, j * HID : (j + 1) * HID],
                        lhsT=xpc[:, j * P : (j + 1) * P],
                        rhs=W1s[:],
                        start=True, stop=True,
                    )
                dis_b = dis[:, b0 : b0 + nb].rearrange("p (b one) -> p b one", one=1)
                nc.vector.tensor_tensor(
                    out=htab[:, b0 * HID : (b0 + nb) * HID],
                    in0=pst[:, : nb * HID].rearrange("p (b f) -> p b f", f=HID),
                    in1=dis_b.to_broadcast([P, nb, HID]),
                    op=mybir.AluOpType.mult,
                )
            nc.sync.dma_start(
                out=ag1in[:].rearrange("(b ph) (pl f) -> (ph pl) b f",
                                       ph=16, pl=8, f=HID),
                in_=htab[:].rearrange("p (b f) -> p b f", f=HID),
            )
            nc.gpsimd.collective_compute(
                "AllGather", mybir.AluOpType.bypass,
                replica_groups=[list(range(NCORES))],
                ins=[ag1in.opt()], outs=[table1.opt()],
            )

            qn = [0]

            def aggregate(table, accname):
                acc = wk.tile([P, FB], f32, name=accname)
                for (b0, b1, s0, s1) in CHUNKS:
                    SC = s1 - s0
                    bt = gth.tile([P, SCMAX * 128], bf16, tag="bt")
                    ixt = ixp.tile([P, SCMAX * 8], i16, tag="ixt")
                    nc.sync.dma_start(out=ixt[:, : SC * 8],
                                      in_=idxg_d[:, s0 * 8 : s1 * 8])
                    mkt = ixp.tile([P, SCMAX * 8], bf16, tag="mkt")
                    nc.sync.dma_start(out=mkt[:, : SC * 8],
                                      in_=maskt_d[:, s0 * 8 : s1 * 8])
                    # gather in <= ICOLS column pieces (descriptor ring limit)
                    for c0 in range(0, SC, ICOLS):
                        c1 = min(c0 + ICOLS, SC)
                        nidx = (c1 - c0) * P
                        nc.gpsimd.dma_gather(
                            out_ap=bt[:, c0 * 128 : c1 * 128].rearrange(
                                "p (s e) -> p s e", e=128
                            ),
                            in_ap=table[:, :],
                            idxs_ap=ixt[:, c0 * 8 : c1 * 8],
                            num_idxs=nidx,
                            num_idxs_reg=nidx,
                            elem_size=128,
                            queue_num=qn[0] % NQ,
                        )
                        qn[0] += 1
                    # select the target node in each gathered 8-group
                    nc.vector.tensor_tensor(
                        out=bt[:, : SC * 128].rearrange(
                            "p (sr f) -> p sr f", f=HID
                        ),
                        in0=bt[:, : SC * 128].rearrange(
                            "p (sr f) -> p sr f", f=HID
                        ),
                        in1=mkt[:, : SC * 8].rearrange(
                            "p (sr one) -> p sr one", one=1
                        ).to_broadcast([P, SC * 8, HID]),
                        op=mybir.AluOpType.mult,
                    )
                    # in-place tree-sum over the 8 lanes: each slot has at
                    # most one nonzero lane, so bf16 sums are exact; the
                    # selected value ends at lane 0 (first 16 cols per slot)
                    for h, e in ((2, 64), (4, 32), (8, 16)):
                        v = bt[:, : SC * 128].rearrange(
                            "p (s h e) -> p s h e", h=h, e=e
                        )
                        nc.vector.tensor_tensor(
                            out=v[:, :, 0:1, :], in0=v[:, :, 0:1, :],
                            in1=v[:, :, 1:2, :], op=mybir.AluOpType.add,
                        )
                    # per-block sum over slot columns (batch equal-K runs)
                    b = b0
                    while b < b1:
                        be = b
                        while be < b1 and K[be] == K[b]:
                            be += 1
                        kb = K[b]
                        nb = be - b
                        o = OFF[b] - s0
                        nc.vector.reduce_sum(
                            out=acc[:, b * HID : be * HID].rearrange(
                                "p (nb f) -> p nb f", f=HID
                            ),
                            in_=bt[:, o * 128 : (o + nb * kb) * 128].rearrange(
                                "p (nb k e) -> p nb e k", k=kb, e=128
                            )[:, :, :HID, :],
                            axis=mybir.AxisListType.X,
                        )
                        b = be
                return acc

            # ---- layer 1 ----
            acc1 = aggregate(table1, "acc1")
            nc.vector.tensor_tensor(out=acc1[:], in0=acc1[:], in1=htab[:],
                                    op=mybir.AluOpType.add)
            # y1 = acc1 * dis ; h1 = relu(y1 + b1) ; htab2 = dis * h1
            nc.vector.tensor_tensor(
                out=acc1[:],
                in0=acc1[:].rearrange("p (b f) -> p b f", f=HID),
                in1=disB.to_broadcast([P, BLKS, HID]),
                op=mybir.AluOpType.mult,
            )
            nc.vector.tensor_tensor(
                out=acc1[:],
                in0=acc1[:].rearrange("p (b f) -> p b f", f=HID),
                in1=b1s[:].rearrange("p (one f) -> p one f", one=1).to_broadcast([P, BLKS, HID]),
                op=mybir.AluOpType.add,
            )
            nc.scalar.activation(out=acc1[:], in_=acc1[:],
                                 func=mybir.ActivationFunctionType.Relu)
            htab2 = wk.tile([P, FB], bf16)
            nc.vector.tensor_tensor(
                out=htab2[:],
                in0=acc1[:].rearrange("p (b f) -> p b f", f=HID),
                in1=disB.to_broadcast([P, BLKS, HID]),
                op=mybir.AluOpType.mult,
            )

            nc.sync.dma_start(
                out=ag2in[:].rearrange("(b ph) (pl f) -> (ph pl) b f",
                                       ph=16, pl=8, f=HID),
                in_=htab2[:].rearrange("p (b f) -> p b f", f=HID),
            )
            nc.gpsimd.collective_compute(
                "AllGather", mybir.AluOpType.bypass,
                replica_groups=[list(range(NCORES))],
                ins=[ag2in.opt()], outs=[table2.opt()],
            )

            # ---- layer 2 ----
            acc2 = aggregate(table2, "acc2")
            nc.vector.tensor_tensor(out=acc2[:], in0=acc2[:], in1=htab2[:],
                                    op=mybir.AluOpType.add)
            nc.vector.tensor_tensor(
                out=acc2[:],
                in0=acc2[:].rearrange("p (b f) -> p b f", f=HID),
                in1=disB.to_broadcast([P, BLKS, HID]),
                op=mybir.AluOpType.mult,
            )
            nc.vector.tensor_tensor(
                out=acc2[:],
                in0=acc2[:].rearrange("p (b f) -> p b f", f=HID),
                in1=W2s[:].rearrange("p (one f) -> p one f", one=1).to_broadcast([P, BLKS, HID]),
                op=mybir.AluOpType.mult,
            )
            y2 = wk.tile([P, BLKS], f32)
            nc.vector.reduce_sum(
                out=y2[:],
                in_=acc2[:].rearrange("p (b f) -> p b f", f=HID),
                axis=mybir.AxisListType.X,
            )
            nc.vector.tensor_tensor(
                out=y2[:],
                in0=y2[:],
                in1=b2s[:].to_broadcast([P, BLKS]),
                op=mybir.AluOpType.add,
            )
            nc.sync.dma_start(
                out=out_d[:].rearrange("(b p) -> p b", p=P),
                in_=y2[:],
            )
    nc.compile()
    return nc


def kernel(x, edge_index, W1, b1, W2, b2):
    global LAST_RESULTS
    _install_shims()
    from concourse.bass_utils import run_bass_kernel_spmd

    in_maps, meta = _host_prep(x, edge_index, W1, b1, W2, b2)
    nc = _build_nc(meta["K"], meta["C_idx"], meta["chunks"])
    res = run_bass_kernel_spmd(
        nc, in_maps, core_ids=list(range(NCORES)),
        trace=bool(os.environ.get("BASS_TRACE")),
    )
    LAST_RESULTS = res
    out = np.empty((N, 1), dtype=np.float32)
    for c in range(NCORES):
        yc = res.results[c]["o"]            # [NPAD], rank-ordered
        lo = c * NSLICE
        out[lo + meta["orders"][c], 0] = yc[:NSLICE]
    return out


# revision 21
# speedup vs baseline: 3.3531x; 1.0177x over previous
"""2-layer GCN (GCNConv x2) on 8 Trainium2 NeuronCores via Bass.

Strategy (dst-sharded, dma_gather ELL):
- Nodes sharded into 8 contiguous slices of 31250 (padded to 31360 = 245*128).
- Within each core, nodes are sorted by in-degree and packed into 245 blocks
  of 128 (block b, partition p). Per-block ELL: K_b slot columns per block
  (max in-degree + 1 self slot, maxed across cores).
- Feature table: dis*h per node, bf16, packed 8 nodes per 256B "group" row
  ([31360 groups, 128 bf16] globally). AllGathered per layer (1MB/rank).
- Gather: InstDMAGatherAnt (gpsimd.dma_gather) with int16 group indices,
  8 ELL columns (1024 idxs / 1024 descriptors) per instruction -- descriptor
  ring limit. Index j lands at [p=j%128, col=j//128], so landing position
  encodes the destination slot; no scatter needed.
- Each gathered 256B group holds 8 candidate nodes; a per-slot one-hot bf16
  mask (8 lanes) selects the right node (and zeroes padding slots). One
  strided DVE reduce per block sums over (slots x 8 lanes).
- Self-loops are ELL slots pointing at the node's own group/residue: with
  A_hat = D^-1/2 (A+I) D^-1/2 factored as pre-scale (table rows carry dis*h)
  and post-scale (dis * aggregate), the self term dis^2*h is exact.
"""
import os
import sys
import types

sys.path.insert(0, "/opt/trn_rl_repo")

import numpy as np
import ml_dtypes

N = 250000
E = 4000000
IN_F, HID, OUT = 18, 16, 1
NCORES = 8
NSLICE = N // NCORES            # 31250
BLKS = (NSLICE + 127) // 128    # 245
NPAD = BLKS * 128               # 31360
P = 128
GSEG = NPAD // 8                # 3920 groups per core segment
G_ALL = NCORES * GSEG           # 31360 groups globally (< int16 max)

SCMAX = 64                      # ELL columns per super-chunk (gather tile)
ICOLS = 8                       # ELL columns per dma_gather instruction
NQ = 4                          # SWDGE queues

LAST_RESULTS = None             # test.py reads exec_time_ns from here


def _install_shims():
    """Make run_bass_kernel_spmd(trace=True) work in this container."""
    try:
        import antenv.axon_hooks  # noqa: F401
    except ImportError:
        import antenv
        mod = types.ModuleType("antenv.axon_hooks")
        _hook = [None]
        mod.set_axon_ntff_profile_hook = lambda h: _hook.__setitem__(0, h)
        mod.get_axon_ntff_profile_hook = lambda: _hook[0]
        sys.modules["antenv.axon_hooks"] = mod
        antenv.axon_hooks = mod
        try:
            from trn_agent_boot import trn_boot
            mod.set_axon_ntff_profile_hook(
                trn_boot._ntff_profile_via_ctypes("/opt/axon/libaxon_pjrt.so")
            )
        except Exception:
            pass
    from concourse import bass_utils
    bass_utils.upload_artifacts = lambda tmpdir: tmpdir


def _host_prep(x, edge_index, W1, b1, W2, b2):
    src = np.asarray(edge_index[0], dtype=np.int64).astype(np.int32)
    dst = np.asarray(edge_index[1], dtype=np.int64).astype(np.int32)
    x = np.asarray(x, dtype=np.float32)

    deg_in = np.bincount(dst, minlength=N).astype(np.int64)   # without self loop

    # per-core degree-ascending rank of each node
    rank = np.empty(N, dtype=np.int64)
    orders = []
    for c in range(NCORES):
        lo, hi = c * NSLICE, (c + 1) * NSLICE
        order = np.argsort(deg_in[lo:hi], kind="stable")      # ascending
        orders.append(order)
        rank[lo + order] = np.arange(NSLICE)
    owner = np.arange(N) // NSLICE
    table_row = owner * NPAD + rank                           # global table row

    # common per-block slot counts K_b (max over cores) + 1 self slot
    K = np.zeros(BLKS, dtype=np.int64)
    for c in range(NCORES):
        lo = c * NSLICE
        ds = deg_in[lo + orders[c]]                           # ascending
        ds_pad = np.concatenate([ds, np.zeros(NPAD - NSLICE, np.int64)])
        K = np.maximum(K, ds_pad.reshape(BLKS, P).max(axis=1))
    K = np.maximum(K, 1)
    off = np.concatenate([[0], np.cumsum(K)]).astype(np.int64)
    C_idx = int(off[-1])

    # place each edge: sorted by dst, k-th in-edge of node d goes to
    # column off[b]+k on partition p, where rank[d] = b*128+p
    es = np.argsort(dst, kind="stable")
    dsts = dst[es]
    srcs = src[es]
    run_first = np.searchsorted(dsts, np.arange(N))           # first pos per node
    k_arr = np.arange(E, dtype=np.int64) - run_first[dsts]
    c_arr = dsts // NSLICE
    r_arr = rank[dsts]
    b_arr = r_arr // P
    p_arr = r_arr % P
    col_arr = off[b_arr] + k_arr
    trow_arr = table_row[srcs]

    # per-slot group index + residue + validity, [NCORES, P, C_idx]
    grp_all = np.zeros((NCORES, P, C_idx), dtype=np.int16)
    res_all = np.zeros((NCORES, P, C_idx), dtype=np.int8)
    val_all = np.zeros((NCORES, P, C_idx), dtype=bool)
    grp_all[c_arr, p_arr, col_arr] = (trow_arr // 8).astype(np.int16)
    res_all[c_arr, p_arr, col_arr] = (trow_arr % 8).astype(np.int8)
    val_all[c_arr, p_arr, col_arr] = True

    # block-aligned super-chunks of <= SCMAX columns
    chunks = []
    cb0 = 0
    while cb0 < BLKS:
        cb1 = cb0
        while cb1 < BLKS and (off[cb1 + 1] - off[cb0]) <= SCMAX:
            cb1 += 1
        assert cb1 > cb0, f"block {cb0} has K={K[cb0]} > SCMAX={SCMAX}"
        chunks.append((cb0, cb1, int(off[cb0]), int(off[cb1])))
        cb0 = cb1

    # per-core tensors
    in_maps = []
    for c in range(NCORES):
        lo = c * NSLICE
        order = orders[c]
        xT = np.zeros((IN_F, NPAD), dtype=np.float32)
        xT[:, :NSLICE] = x[lo + order].T
        deg_t = np.ones(NPAD, dtype=np.float32)
        deg_t[:NSLICE] = deg_in[lo + order].astype(np.float32) + 1.0
        deg_t = deg_t.reshape(BLKS, P).T.copy()               # [128, 245]

        # idx wrap: element j = s*128+p of any column-aligned instruction is
        # at [j%16, s*8 + p//16]; replicated across the 8 16-partition groups
        idxg = (
            grp_all[c].reshape(8, 16, C_idx).transpose(1, 2, 0).reshape(16, C_idx * 8)
        )
        idxg = np.tile(idxg, (8, 1)).astype(np.int16)

        # mask: [p, s*8 + r] one-hot bf16 (zero for padding slots)
        onehot = (
            np.arange(8, dtype=np.int8)[None, None, :] == res_all[c][:, :, None]
        ) & val_all[c][:, :, None]
        maskt = onehot.reshape(P, C_idx * 8).astype(ml_dtypes.bfloat16)

        in_maps.append({
            "xT": xT,
            "degt": deg_t,
            "idxg": idxg,
            "maskt": maskt,
            "W1": np.asarray(W1, dtype=np.float32),
            "W2r": np.tile(np.asarray(W2, np.float32).reshape(1, HID), (P, 1)),
            "b1r": np.tile(np.asarray(b1, np.float32).reshape(1, HID), (P, 1)),
            "b2r": np.full((P, 1), np.float32(np.asarray(b2).reshape(-1)[0])),
        })
    meta = {"K": K.tolist(), "off": off.tolist(), "C_idx": C_idx,
            "orders": orders, "chunks": chunks}
    return in_maps, meta


def _build_nc(K, C_idx, CHUNKS):
    from concourse import bass, bacc, mybir
    import concourse.tile as tile

    nc = bacc.Bacc("TRN2", target_bir_lowering=False, debug=False,
                   num_devices=NCORES, num_swdge_queues=NQ)
    f32 = mybir.dt.float32
    bf16 = mybir.dt.bfloat16
    i16 = mybir.dt.int16
    xT_d = nc.dram_tensor("xT", [IN_F, NPAD], f32, kind="ExternalInput")
    degt_d = nc.dram_tensor("degt", [P, BLKS], f32, kind="ExternalInput")
    idxg_d = nc.dram_tensor("idxg", [P, C_idx * 8], i16, kind="ExternalInput")
    maskt_d = nc.dram_tensor("maskt", [P, C_idx * 8], bf16, kind="ExternalInput")
    W1_d = nc.dram_tensor("W1", [IN_F, HID], f32, kind="ExternalInput")
    W2r_d = nc.dram_tensor("W2r", [P, HID], f32, kind="ExternalInput")
    b1r_d = nc.dram_tensor("b1r", [P, HID], f32, kind="ExternalInput")
    b2r_d = nc.dram_tensor("b2r", [P, 1], f32, kind="ExternalInput")
    out_d = nc.dram_tensor("o", [NPAD], f32, kind="ExternalOutput")

    FB = BLKS * HID  # 3920 free cols for [p, (b f)] layouts

    OFF = [0]
    for kb in K:
        OFF.append(OFF[-1] + kb)

    with tile.TileContext(nc) as tc:
        with (
            tc.tile_pool(name="const", bufs=1) as cp,
            tc.tile_pool(name="xp", bufs=2) as xp,
            tc.tile_pool(name="ps", bufs=2, space="PSUM") as psp,
            tc.tile_pool(name="wk", bufs=1) as wk,
            tc.tile_pool(name="gth", bufs=6) as gth,
            tc.tile_pool(name="ixp", bufs=6) as ixp,
            tc.tile_pool(name="dram", bufs=1, space="DRAM") as dr,
        ):
            W1s = cp.tile([IN_F, HID], f32)
            nc.sync.dma_start(out=W1s[:], in_=W1_d[:])
            W2s = cp.tile([P, HID], f32)
            nc.sync.dma_start(out=W2s[:], in_=W2r_d[:])
            b1s = cp.tile([P, HID], f32)
            nc.sync.dma_start(out=b1s[:], in_=b1r_d[:])
            b2s = cp.tile([P, 1], f32)
            nc.sync.dma_start(out=b2s[:], in_=b2r_d[:])
            degs = cp.tile([P, BLKS], f32)
            nc.sync.dma_start(out=degs[:], in_=degt_d[:])

            dis = cp.tile([P, BLKS], f32)
            nc.vector.reciprocal(out=dis[:], in_=degs[:])
            nc.scalar.activation(out=dis[:], in_=dis[:],
                                 func=mybir.ActivationFunctionType.Sqrt)

            ag1in = dr.tile([GSEG, 128], bf16)
            table1 = dr.tile([G_ALL, 128], bf16, addr_space="Shared")
            ag2in = dr.tile([GSEG, 128], bf16)
            table2 = dr.tile([G_ALL, 128], bf16, addr_space="Shared")

            disB = dis[:].rearrange("p (b one) -> p b one", one=1)

            # ---- phase A: htab = dis * (x @ W1) in bf16, rank order ----
            htab = wk.tile([P, FB], bf16)
            CHUNK = 16
            for piece in range((BLKS + CHUNK - 1) // CHUNK):
                b0 = piece * CHUNK
                nb = min(CHUNK, BLKS - b0)
                xpc = xp.tile([IN_F, CHUNK * P], f32, tag="xpc")
                nc.sync.dma_start(out=xpc[:, : nb * P],
                                  in_=xT_d[:, b0 * P : (b0 + nb) * P])
                pst = psp.tile([P, CHUNK * HID], f32, tag="pst")
                for j in range(nb):
                    nc.tensor.matmul(
                        out=pst[:, j * HID : (j + 1) * HID],
                        lhsT=xpc[:, j * P : (j + 1) * P],
                        rhs=W1s[:],
                        start=True, stop=True,
                    )
                dis_b = dis[:, b0 : b0 + nb].rearrange("p (b one) -> p b one", one=1)
                nc.vector.tensor_tensor(
                    out=htab[:, b0 * HID : (b0 + nb) * HID],
                    in0=pst[:, : nb * HID].rearrange("p (b f) -> p b f", f=HID),
                    in1=dis_b.to_broadcast([P, nb, HID]),
                    op=mybir.AluOpType.mult,
                )
            nc.sync.dma_start(
                out=ag1in[:].rearrange("(b ph) (pl f) -> (ph pl) b f",
                                       ph=16, pl=8, f=HID),
                in_=htab[:].rearrange("p (b f) -> p b f", f=HID),
            )
            nc.gpsimd.collective_compute(
                "AllGather", mybir.AluOpType.bypass,
                replica_groups=[list(range(NCORES))],
                ins=[ag1in.opt()], outs=[table1.opt()],
            )

            qn = [0]

            def aggregate(table, accname):
                acc = wk.tile([P, FB], f32, name=accname)
                for (b0, b1, s0, s1) in CHUNKS:
                    SC = s1 - s0
                    bt = gth.tile([P, SCMAX * 128], bf16, tag="bt")
                    ixt = ixp.tile([P, SCMAX * 8], i16, tag="ixt")
                    nc.sync.dma_start(out=ixt[:, : SC * 8],
                                      in_=idxg_d[:, s0 * 8 : s1 * 8])
                    mkt = ixp.tile([P, SCMAX * 8], bf16, tag="mkt")
                    nc.sync.dma_start(out=mkt[:, : SC * 8],
                                      in_=maskt_d[:, s0 * 8 : s1 * 8])
                    # gather in <= ICOLS column pieces (descriptor ring limit)
                    for c0 in range(0, SC, ICOLS):
                        c1 = min(c0 + ICOLS, SC)
                        nidx = (c1 - c0) * P
                        nc.gpsimd.dma_gather(
                            out_ap=bt[:, c0 * 128 : c1 * 128].rearrange(
                                "p (s e) -> p s e", e=128
                            ),
                            in_ap=table[:, :],
                            idxs_ap=ixt[:, c0 * 8 : c1 * 8],
                            num_idxs=nidx,
                            num_idxs_reg=nidx,
                            elem_size=128,
                            queue_num=qn[0] % NQ,
                        )
                        qn[0] += 1
                    # select the target node in each gathered 8-group
                    nc.vector.tensor_tensor(
                        out=bt[:, : SC * 128].rearrange(
                            "p (sr f) -> p sr f", f=HID
                        ),
                        in0=bt[:, : SC * 128].rearrange(
                            "p (sr f) -> p sr f", f=HID
                        ),
                        in1=mkt[:, : SC * 8].rearrange(
                            "p (sr one) -> p sr one", one=1
                        ).to_broadcast([P, SC * 8, HID]),
                        op=mybir.AluOpType.mult,
                    )
                    # in-place tree-sum over the 8 lanes: each slot has at
                    # most one nonzero lane, so bf16 sums are exact; the
                    # selected value ends at lane 0 (first 16 cols per slot)
                    for h, e in ((2, 64), (4, 32), (8, 16)):
                        v = bt[:, : SC * 128].rearrange(
                            "p (s h e) -> p s h e", h=h, e=e
                        )
                        nc.vector.tensor_tensor(
                            out=v[:, :, 0:1, :], in0=v[:, :, 0:1, :],
                            in1=v[:, :, 1:2, :], op=mybir.AluOpType.add,
                        )
                    # per-block sum over slot columns (batch equal-K runs)
                    b = b0
                    while b < b1:
                        be = b
                        while be < b1 and K[be] == K[b]:
                            be += 1
                        kb = K[b]
                        nb = be - b
                        o = OFF[b] - s0
                        nc.vector.reduce_sum(
                            out=acc[:, b * HID : be * HID].rearrange(
                                "p (nb f) -> p nb f", f=HID
                            ),
                            in_=bt[:, o * 128 : (o + nb * kb) * 128].rearrange(
                                "p (nb k e) -> p nb e k", k=kb, e=128
                            )[:, :, :HID, :],
                            axis=mybir.AxisListType.X,
                        )
                        b = be
                return acc

            # ---- layer 1 ----
            acc1 = aggregate(table1, "acc1")
            nc.vector.tensor_tensor(out=acc1[:], in0=acc1[:], in1=htab[:],
                                    op=mybir.AluOpType.add)
            # y1 = acc1 * dis ; h1 = relu(y1 + b1) ; htab2 = dis * h1
            nc.vector.tensor_tensor(
                out=acc1[:],
                in0=acc1[:].rearrange("p (b f) -> p b f", f=HID),
                in1=disB.to_broadcast([P, BLKS, HID]),
                op=mybir.AluOpType.mult,
            )
            nc.vector.tensor_tensor(
                out=acc1[:],
                in0=acc1[:].rearrange("p (b f) -> p b f", f=HID),
                in1=b1s[:].rearrange("p (one f) -> p one f", one=1).to_broadcast([P, BLKS, HID]),
                op=mybir.AluOpType.add,
            )
            nc.scalar.activation(out=acc1[:], in_=acc1[:],
                                 func=mybir.ActivationFunctionType.Relu)
            htab2 = wk.tile([P, FB], bf16)
            nc.vector.tensor_tensor(
                out=htab2[:],
                in0=acc1[:].rearrange("p (b f) -> p b f", f=HID),
                in1=disB.to_broadcast([P, BLKS, HID]),
                op=mybir.AluOpType.mult,
            )

            nc.sync.dma_start(
                out=ag2in[:].rearrange("(b ph) (pl f) -> (ph pl) b f",
                                       ph=16, pl=8, f=HID),
                in_=htab2[:].rearrange("p (b f) -> p b f", f=HID),
            )
            nc.gpsimd.collective_compute(
                "AllGather", mybir.AluOpType.bypass,
                replica_groups=[list(range(NCORES))],
                ins=[ag2in.opt()], outs=[table2.opt()],
            )

            # ---- layer 2 ----
            acc2 = aggregate(table2, "acc2")
            nc.vector.tensor_tensor(out=acc2[:], in0=acc2[:], in1=htab2[:],
                                    op=mybir.AluOpType.add)
            nc.vector.tensor_tensor(
                out=acc2[:],
                in0=acc2[:].rearrange("p (b f) -> p b f", f=HID),
                in1=disB.to_broadcast([P, BLKS, HID]),
                op=mybir.AluOpType.mult,
            )
            nc.vector.tensor_tensor(
                out=acc2[:],
                in0=acc2[:].rearrange("p (b f) -> p b f", f=HID),
                in1=W2s[:].rearrange("p (one f) -> p one f", one=1).to_broadcast([P, BLKS, HID]),
                op=mybir.AluOpType.mult,
            )
            y2 = wk.tile([P, BLKS], f32)
            nc.vector.reduce_sum(
                out=y2[:],
                in_=acc2[:].rearrange("p (b f) -> p b f", f=HID),
                axis=mybir.AxisListType.X,
            )
            nc.vector.tensor_tensor(
                out=y2[:],
                in0=y2[:],
                in1=b2s[:].to_broadcast([P, BLKS]),
                op=mybir.AluOpType.add,
            )
            nc.sync.dma_start(
                out=out_d[:].rearrange("(b p) -> p b", p=P),
                in_=y2[:],
            )
    nc.compile()
    return nc


def kernel(x, edge_index, W1, b1, W2, b2):
    global LAST_RESULTS
    _install_shims()
    from concourse.bass_utils import run_bass_kernel_spmd

    in_maps, meta = _host_prep(x, edge_index, W1, b1, W2, b2)
    nc = _build_nc(meta["K"], meta["C_idx"], meta["chunks"])
    res = run_bass_kernel_spmd(
        nc, in_maps, core_ids=list(range(NCORES)),
        trace=bool(os.environ.get("BASS_TRACE")),
    )
    LAST_RESULTS = res
    out = np.empty((N, 1), dtype=np.float32)
    for c in range(NCORES):
        yc = res.results[c]["o"]            # [NPAD], rank-ordered
        lo = c * NSLICE
        out[lo + meta["orders"][c], 0] = yc[:NSLICE]
    return out
